# revision 6
# baseline (speedup 1.0000x reference)
"""Multi-head attention (B=2,S=2048,D=1024,H=16) on 8 trn2 NeuronCores.

Sharding: core = b*4 + g  (b = batch 0..1, g = head-group 0..3, 4 heads each).
Each core computes QKV projections for its 256 output dims, causal attention
for its 4 heads (scores kept transposed: [s_k, s_q]), and a K-sliced partial
of the output projection (transposed: [D, S]).  Host sums the 4 partials per
batch and adds b_o.

All matmuls in bf16 (fp32 PSUM accumulate); softmax without max-subtraction
(scores/8 are small, exp cannot overflow).  The PV stationary for head h is
[vp_h | ones] (128 wide): out rows 0:64 accumulate P@V, rows 64:128
accumulate sum(P) broadcast 64-wide -- sumexp costs zero extra PE streams.
Normalization is reciprocal + elementwise multiply on DVE.  v-bias is added
by the DVE copy that moves the v-projection psum into the [vp|ones] layout.
"""
import sys

if "/opt/trn_rl_repo" not in sys.path:
    sys.path.insert(0, "/opt/trn_rl_repo")

import numpy as np
import ml_dtypes

B, S, D, H = 2, 2048, 1024, 16
HD = D // H            # 64
G = 4                  # head groups (one per core within a batch)
HPG = H // G           # 4 heads per group
DG = HPG * HD          # 256 dims per group
SCALE = 8.0
NCORES = 8
NQC = S // 512         # 4 query chunks
NJ = S // 128          # 16 key tiles
KC = D // 128          # 8 contraction chunks
BF16 = ml_dtypes.bfloat16

_CACHE = {}


def _build(causal: bool):
    import concourse.mybir as mybir
    import concourse.tile as tile
    from concourse import bacc

    f32 = mybir.dt.float32
    b16 = mybir.dt.bfloat16
    Exp = mybir.ActivationFunctionType.Exp

    nc = bacc.Bacc(None, target_bir_lowering=False)

    qT = nc.dram_tensor("qT", [D, S], b16, kind="ExternalInput")
    kT = nc.dram_tensor("kT", [D, S], b16, kind="ExternalInput")
    vT = nc.dram_tensor("vT", [D, S], b16, kind="ExternalInput")
    # weights host-prepacked to the exact SBUF tile layout (one DMA each)
    # wq/wk: m-major [128, m*D + kc*128] so the m=0 projection can start
    # after only the first half of the weight lands
    wqT = nc.dram_tensor("wqT", [128, 2 * D], b16, kind="ExternalInput")
    wkT = nc.dram_tensor("wkT", [128, 2 * D], b16, kind="ExternalInput")
    wvT = nc.dram_tensor("wvT", [128, KC * DG], b16, kind="ExternalInput")
    woT = nc.dram_tensor("woT", [128, 2 * D], b16, kind="ExternalInput")
    bq = nc.dram_tensor("bq", [128, 2], f32, kind="ExternalInput")
    bk = nc.dram_tensor("bk", [128, 2], f32, kind="ExternalInput")
    bvb = nc.dram_tensor("bvb", [128, DG], b16, kind="ExternalInput")
    tri = nc.dram_tensor("tri", [128, 128], b16, kind="ExternalInput")
    out = nc.dram_tensor("out", [D, S], b16, kind="ExternalOutput")

    with tile.TileContext(nc) as tc:
        with (
            tc.tile_pool(name="consts", bufs=1) as consts,
            tc.tile_pool(name="proj", bufs=1) as proj,
            tc.tile_pool(name="pin", bufs=1) as pin,
            tc.tile_pool(name="probs", bufs=8) as probsp,
            tc.tile_pool(name="rec", bufs=2) as recp,
            tc.tile_pool(name="ost", bufs=3) as ostp,
            tc.tile_pool(name="mm", bufs=3, space="PSUM") as mmp,
            tc.tile_pool(name="cu", bufs=1, space="PSUM") as cup,
        ):
            # --- constant tiles -------------------------------------------
            wq_t = consts.tile([128, 2 * D], b16)
            wk_t = consts.tile([128, 2 * D], b16)
            wv_t = consts.tile([128, KC * DG], b16)
            wo_t = consts.tile([128, 2 * D], b16)
            bq_t = consts.tile([128, 2], f32)
            bk_t = consts.tile([128, 2], f32)
            bvb_t = consts.tile([128, DG], b16)
            tri_t = consts.tile([128, 128], b16)
            warm_sb = consts.tile([128, 128], b16)
            nc.vector.memset(warm_sb[:], 0.0)

            # --- persistent projection outputs ----------------------------
            # qpT/kpT: pair p in cols [p*S,(p+1)*S); rows 0:64 head 2p, 64:128 head 2p+1
            qpT = proj.tile([128, 2 * S], b16)
            kpT = proj.tile([128, 2 * S], b16)
            # vp: key tile j at cols [j*512,(j+1)*512); head h at +h*128:
            # cols 0:64 = projected v, cols 64:128 = 1.0 (sumexp column block)
            vp = proj.tile([128, NJ * 512], b16)
            nc.vector.memset(vp[:], 1.0)
            # ctxT: same pair layout as qpT, normalized attention output (c x s)
            ctxT = proj.tile([128, 2 * S], b16)

            # --- input row tiles + DMA schedule ---------------------------
            # quarter-split column DMAs so compute starts as soon as the
            # first 512 columns of each row land
            qrow = [pin.tile([128, S], b16, name=f"qrow{kc}") for kc in range(KC)]
            krow = [pin.tile([128, S], b16, name=f"krow{kc}") for kc in range(KC)]
            vrow = [pin.tile([128, S], b16, name=f"vrow{kc}") for kc in range(KC)]

            def dma_rows(rows, src, cs):
                for kc in range(KC):
                    nc.sync.dma_start(rows[kc][:, cs], src[kc * 128:(kc + 1) * 128, cs])

            def dma_w4(w_t, src_w):
                # 4 parallel chunks so the weight lands ~4x sooner
                for ch in range(4):
                    cs = slice(ch * 512, (ch + 1) * 512)
                    nc.gpsimd.dma_start(w_t[:, cs], src_w[:, cs])

            q0, q1, half1 = slice(0, 512), slice(512, 1024), slice(1024, 2048)
            # weights on the gpsimd DMA queue, input rows on the sync queue:
            # two descriptor issuers in parallel so first bytes land sooner.
            # chunk 0 needs only the first 512 columns of q/k/v, so those
            # stream first.
            for ch in range(4):
                cs = slice(ch * 512, (ch + 1) * 512)
                nc.gpsimd.dma_start(wq_t[:, cs], wqT[:, cs])
            dma_rows(qrow, qT, q0)
            for ch in range(4):
                cs = slice(ch * 512, (ch + 1) * 512)
                nc.gpsimd.dma_start(wk_t[:, cs], wkT[:, cs])
            nc.gpsimd.dma_start(bq_t[:], bq[:])
            nc.gpsimd.dma_start(bk_t[:], bk[:])
            nc.gpsimd.dma_start(bvb_t[:], bvb[:])
            nc.gpsimd.dma_start(tri_t[:], tri[:])
            dma_rows(krow, kT, q0)
            dma_w4(wv_t, wvT)
            dma_rows(vrow, vT, q0)
            dma_w4(wo_t, woT)
            dma_rows(qrow, qT, q1)
            dma_rows(krow, kT, q1)
            dma_rows(vrow, vT, q1)
            dma_rows(qrow, qT, half1)
            dma_rows(krow, kT, half1)
            dma_rows(vrow, vT, half1)

            # warmup burst: keeps the PE activity monitor at full clock
            # while the first input quarters stream in
            warm_ps = mmp.tile([128, 1024], f32, tag="sc", name="warm")
            for wi in range(48):
                nc.tensor.matmul(warm_ps[:, 0:128], warm_sb[:], warm_sb[:],
                                 start=(wi == 0), stop=(wi == 47))
            # preload the exp spline tables (~2.7us) during the DMA window
            nc.scalar.activation(warm_sb[:, 0:1], warm_sb[:, 0:1], Exp)

            # --- projections interleaved with attention, per quarter ------
            # (single psum pool assigns slots in emission order, so program
            # order must follow the dataflow for cross-phase overlap)

            def qk_proj_m(name, rows, w_t, dst, bias_t, n, m):
                ps = mmp.tile([128, 1024], f32, tag="sc", name=f"{name}ps{m}{n}")[:, 0:512]
                for kc in range(KC):
                    nc.tensor.matmul(
                        ps,
                        w_t[:, m * D + kc * 128: m * D + (kc + 1) * 128],
                        rows[kc][:, n * 512:(n + 1) * 512],
                        start=(kc == 0), stop=(kc == KC - 1),
                    )
                nc.vector.tensor_scalar_add(
                    dst[:, m * S + n * 512: m * S + (n + 1) * 512],
                    ps, bias_t[:, m:m + 1],
                )

            def qk_proj(name, rows, w_t, dst, bias_t, n):
                for m in range(2):
                    qk_proj_m(name, rows, w_t, dst, bias_t, n, m)

            def v_proj_j(j):
                for j in [j]:
                    ps = mmp.tile([128, 1024], f32, tag="sc", name=f"vps{j}")[:, 0:DG]
                    for kc in range(KC):
                        nc.tensor.matmul(
                            ps, vrow[kc][:, j * 128:(j + 1) * 128],
                            wv_t[:, kc * DG:(kc + 1) * DG],
                            start=(kc == 0), stop=(kc == KC - 1),
                        )
                    # copy into the per-head 128-wide slots, adding the v
                    # bias: even heads [vp | ones] (ctx rows 0:64), odd heads
                    # [ones | vp] (ctx rows 64:128) so each head's ctx lands
                    # on the ctxT lanes it must be written to
                    for h in range(HPG):
                        off = 0 if h % 2 == 0 else HD
                        nc.vector.tensor_add(
                            vp[:, j * 512 + h * 128 + off: j * 512 + h * 128 + off + HD],
                            ps[:, h * HD:(h + 1) * HD],
                            bvb_t[:, h * HD:(h + 1) * HD],
                        )

            def v_proj(n):
                for j in range(4 * n, 4 * n + 4):
                    v_proj_j(j)

            def attn_j_sc(c, p, j, nj):
                qoff = p * S + c * 512
                d = j - 4 * c if causal else -1
                coff = 0 if d < 0 else 128 * d
                sc = mmp.tile([128, 1024], f32, tag="sc", name=f"sc{c}{p}{j}")
                for hh, (rlo, rhi) in enumerate(((0, 64), (64, 128))):
                    nc.tensor.matmul(
                        sc[:, hh * 512 + coff: hh * 512 + 512],
                        kpT[rlo:rhi, p * S + j * 128: p * S + (j + 1) * 128],
                        qpT[rlo:rhi, qoff + coff: qoff + 512],
                        start=True, stop=True, tile_position=(rlo, 0),
                    )
                pr = probsp.tile([128, 1024], b16, tag="pr", name=f"pr{c}{p}{j}")
                if coff == 0:
                    nc.scalar.activation(pr[:, 0:1024], sc[:, 0:1024], Exp, scale=1.0 / SCALE)
                else:
                    sc_v = sc.rearrange("p (h n) -> p h n", h=2)[:, :, coff:512]
                    pr_v = pr.rearrange("p (h n) -> p h n", h=2)[:, :, coff:512]
                    nc.scalar.activation(pr_v, sc_v, Exp, scale=1.0 / SCALE)
                if d >= 0:
                    for hh in range(2):
                        band = pr[:, hh * 512 + coff: hh * 512 + coff + 128]
                        nc.vector.tensor_mul(band, band, tri_t[:])
                return pr

            def attn_j_pv(c, p, j, nj, hps, pr):
                d = j - 4 * c if causal else -1
                coff = 0 if d < 0 else 128 * d
                first, last = (j == 0), (j == nj - 1)
                for hh in range(2):
                    h = 2 * p + hh
                    nc.tensor.matmul(
                        hps[:, hh * 512 + coff: hh * 512 + 512],
                        vp[:, j * 512 + h * 128: j * 512 + (h + 1) * 128],
                        pr[:, hh * 512 + coff: hh * 512 + 512],
                        start=first, stop=last, skip_group_check=True,
                    )

            def attn_j(c, p, j, nj, hps):
                pr = attn_j_sc(c, p, j, nj)
                attn_j_pv(c, p, j, nj, hps, pr)

            def attn_pair(c, p, nj, j_lo, j_hi, hps, bg=None, every=2):
                # bg: list of zero-arg emitters (projection groups) woven
                # between attention slots so their psum-slot turns come up
                # mid-stream and their matmuls fill PE slack under the
                # ACT-paced softmax
                bg = list(bg or [])
                k = 0
                for j in range(j_lo, j_hi):
                    attn_j(c, p, j, nj, hps)
                    k += 1
                    if bg and k % every == 0:
                        bg.pop(0)()
                while bg:
                    bg.pop(0)()

            def norm_pair(c, p, hps):
                # hps cols 0:512 = even head (ctx rows 0:64, sum rows
                # 64:128); cols 512:1024 = odd head (sum rows 0:64, ctx rows
                # 64:128).  DVE lanes are independent pipelines, so the
                # reciprocal runs on the sum's own lanes and a tiny
                # SBUF->SBUF DMA moves it across lanes to line up with ctx.
                # reciprocal_approx_fast only works at partition offset 0,
                # and GpSimd (the only cross-lane engine) has no PSUM port:
                # stage sums to SBUF lane-aligned, cross-lane via gpsimd
                # copy, reciprocal at offset 0.
                st = recp.tile([128, 1024], f32, tag="st", name=f"st{c}{p}")
                ra = recp.tile([128, 1024], f32, tag="ra", name=f"ra{c}{p}")
                nc.vector.reciprocal_approx_fast(st[0:64, 512:1024], hps[0:64, 512:1024])
                nc.vector.tensor_copy(st[64:128, 0:512], hps[64:128, 0:512])
                nc.gpsimd.tensor_copy(ra[0:64, 0:512], st[64:128, 0:512])
                nc.gpsimd.tensor_copy(ra[64:128, 512:1024], st[0:64, 512:1024])
                nc.vector.reciprocal_approx_fast(ra[0:64, 0:512], ra[0:64, 0:512])
                nc.vector.tensor_mul(
                    ctxT[0:64, p * S + c * 512: p * S + (c + 1) * 512],
                    hps[0:64, 0:512], ra[0:64, 0:512])
                nc.vector.tensor_mul(
                    ctxT[64:128, p * S + c * 512: p * S + (c + 1) * 512],
                    hps[64:128, 512:1024], ra[64:128, 512:1024])

            def oproj_dc(c, dc):
                for dc in [dc]:
                    ops = mmp.tile([128, 1024], f32, tag="sc", name=f"op{c}{dc}")[:, 0:512]
                    for p2 in range(2):
                        nc.tensor.matmul(
                            ops,
                            wo_t[:, p2 * D + dc * 128: p2 * D + (dc + 1) * 128],
                            ctxT[:, p2 * S + c * 512: p2 * S + (c + 1) * 512],
                            start=(p2 == 0), stop=(p2 == 1),
                        )
                    ot = ostp.tile([128, 512], b16, tag="ot", name=f"ot{c}{dc}")
                    nc.vector.tensor_copy(ot[:], ops)
                    nc.sync.dma_start(
                        out[dc * 128:(dc + 1) * 128, c * 512:(c + 1) * 512], ot[:])

            def oproj(c, dcs=range(KC)):
                for dc in dcs:
                    oproj_dc(c, dc)

            qk_proj("q", qrow, wq_t, qpT, bq_t, 0)
            if not causal:
                # no diagonal structure to pipeline against: project all
                # quarters upfront
                for n in range(NQC):
                    if n > 0:
                        qk_proj("q", qrow, wq_t, qpT, bq_t, n)
                    qk_proj("k", krow, wk_t, kpT, bk_t, n)
                    v_proj(n)
            for c in range(NQC):
                nj = 4 * c + 4 if causal else NJ
                hps0 = cup.tile([128, 1024], f32, tag="cu", name=f"cu{c}0")
                # part A (earlier-quarter key tiles; needs only qpT of this
                # quarter) with this quarter's k/v projections woven between
                # slots so they retire before the diagonal part B needs them
                bg_a = []
                if causal:
                    bg_a += [lambda m=m: qk_proj_m("k", krow, wk_t, kpT, bk_t, c, m) for m in range(2)]
                    bg_a += [lambda j=j: v_proj_j(j) for j in range(4 * c, 4 * c + 4)]
                attn_pair(c, 0, nj, 0, min(4 * c, nj), hps0, bg=bg_a, every=1)
                if c == 0:
                    prs = [attn_j_sc(0, 0, j, nj) for j in range(nj)]
                    for j in range(nj):
                        attn_j_pv(0, 0, j, nj, hps0, prs[j])
                else:
                    attn_pair(c, 0, nj, min(4 * c, nj), nj, hps0)
                norm_pair(c, 0, hps0)
                # pair 1 hosts: previous chunk's output projection and the
                # next quarter's q projection
                bg_b = []
                if c > 0:
                    bg_b += [lambda dc=dc: oproj_dc(c - 1, dc) for dc in range(KC)]
                if causal and c + 1 < NQC:
                    bg_b += [lambda m=m: qk_proj_m("q", qrow, wq_t, qpT, bq_t, c + 1, m) for m in range(2)]
                hps1 = cup.tile([128, 1024], f32, tag="cu", name=f"cu{c}1")
                attn_pair(c, 1, nj, 0, nj, hps1, bg=bg_b, every=2)
                norm_pair(c, 1, hps1)
            oproj(NQC - 1)

    nc.compile()
    return nc


def _get_nc(causal: bool):
    if causal not in _CACHE:
        _CACHE[causal] = _build(causal)
    return _CACHE[causal]


def _pack_w(w):
    # [D, DG] -> SBUF layout [128, KC*DG]: chunk kc of 128 rows side by side
    return np.ascontiguousarray(w.reshape(KC, 128, DG).transpose(1, 0, 2).reshape(128, KC * DG)).astype(BF16)


def _pack_w_mmajor(w):
    # [D, DG] -> SBUF layout [128, 2*D]: half m of the 256 out dims at
    # cols m*D, within that chunk kc at +kc*128
    return np.ascontiguousarray(
        w.reshape(KC, 128, 2, 128).transpose(1, 2, 0, 3).reshape(128, 2 * D)).astype(BF16)


def make_in_maps(q, k, v, w_q, b_q, w_k, b_k, w_v, b_v, w_o):
    tri_keep = (np.arange(128)[:, None] <= np.arange(128)[None, :]).astype(BF16)
    qT = [np.ascontiguousarray(q[b].T).astype(BF16) for b in range(B)]
    kTn = [np.ascontiguousarray(k[b].T).astype(BF16) for b in range(B)]
    vTn = [np.ascontiguousarray(v[b].T).astype(BF16) for b in range(B)]
    in_maps = []
    for core in range(NCORES):
        b, g = core // G, core % G
        sl = slice(g * DG, (g + 1) * DG)
        woTg = np.ascontiguousarray(w_o[:, sl].T)  # [DG, D]
        in_maps.append({
            "qT": qT[b], "kT": kTn[b], "vT": vTn[b],
            "wqT": _pack_w_mmajor(np.ascontiguousarray(w_q[sl, :].T)),
            "wkT": _pack_w_mmajor(np.ascontiguousarray(w_k[sl, :].T)),
            "wvT": _pack_w(np.ascontiguousarray(w_v[sl, :].T)),
            "woT": np.ascontiguousarray(
                woTg.reshape(2, 128, D).transpose(1, 0, 2).reshape(128, 2 * D)).astype(BF16),
            "bq": np.ascontiguousarray(b_q[sl].reshape(2, 128).T).astype(np.float32),
            "bk": np.ascontiguousarray(b_k[sl].reshape(2, 128).T).astype(np.float32),
            "bvb": np.ascontiguousarray(np.broadcast_to(b_v[sl], (128, DG))).astype(BF16),
            "tri": tri_keep,
        })
    return in_maps


def _reference_numpy(q, k, v, mask, w_q, b_q, w_k, b_k, w_v, b_v, w_o, b_o):
    qp = q @ w_q.T + b_q
    kp = k @ w_k.T + b_k
    vv = v @ w_v.T + b_v
    qp = qp.reshape(B, S, H, HD).transpose(0, 2, 1, 3)
    kp = kp.reshape(B, S, H, HD).transpose(0, 2, 1, 3)
    vv = vv.reshape(B, S, H, HD).transpose(0, 2, 1, 3)
    score = np.einsum("bhqd,bhkd->bhqk", qp, kp) / SCALE
    score = np.where(mask, -1e9, score)
    score -= score.max(axis=-1, keepdims=True)
    e = np.exp(score)
    attn = e / e.sum(axis=-1, keepdims=True)
    ctx = np.einsum("bhqk,bhkd->bhqd", attn, vv)
    ctx = ctx.transpose(0, 2, 1, 3).reshape(B, S, D)
    return (ctx @ w_o.T + b_o).astype(np.float32)


def kernel(q, k, v, mask, w_q, b_q, w_k, b_k, w_v, b_v, w_o, b_o):
    from concourse.bass_utils import run_bass_kernel_spmd

    q, k, v = (np.asarray(x, np.float32) for x in (q, k, v))
    mask = np.asarray(mask)
    causal_ref = np.triu(np.ones((S, S), bool), k=1)
    causal = all(np.array_equal(mask[b, 0], causal_ref) for b in range(B))
    if not causal and mask.any():
        # Unexpected mask pattern: fall back to exact numpy (never hit in
        # practice -- setup_inputs always builds the causal mask).
        return _reference_numpy(q, k, v, mask, w_q, b_q, w_k, b_k, w_v, b_v, w_o, b_o)

    nc = _get_nc(causal)
    in_maps = make_in_maps(q, k, v, w_q, b_q, w_k, b_k, w_v, b_v, w_o)
    res = run_bass_kernel_spmd(nc, in_maps, core_ids=list(range(NCORES)))

    out = np.zeros((B, S, D), np.float32)
    for core in range(NCORES):
        b = core // G
        out[b] += res.results[core]["out"].T.astype(np.float32)
    out += np.asarray(b_o, np.float32)
    return out


# revision 13
# speedup vs baseline: 1.0427x; 1.0427x over previous
"""Multi-head attention (B=2,S=2048,D=1024,H=16) on 8 trn2 NeuronCores.

Sharding: core = b*4 + g  (b = batch 0..1, g = head-group 0..3, 4 heads each).
Each core computes QKV projections for its 256 output dims, causal attention
for its 4 heads (scores kept transposed: [s_k, s_q]), and a K-sliced partial
of the output projection (transposed: [D, S]).  Host sums the 4 partials per
batch and adds b_o.

All matmuls in bf16 (fp32 PSUM accumulate); softmax without max-subtraction
(scores/8 are small, exp cannot overflow).  The PV stationary for head h is
[vp_h | ones] (128 wide): out rows 0:64 accumulate P@V, rows 64:128
accumulate sum(P) broadcast 64-wide -- sumexp costs zero extra PE streams.
Normalization is reciprocal + elementwise multiply on DVE.  v-bias is added
by the DVE copy that moves the v-projection psum into the [vp|ones] layout.
"""
import sys

if "/opt/trn_rl_repo" not in sys.path:
    sys.path.insert(0, "/opt/trn_rl_repo")

import numpy as np
import ml_dtypes

B, S, D, H = 2, 2048, 1024, 16
HD = D // H            # 64
G = 4                  # head groups (one per core within a batch)
HPG = H // G           # 4 heads per group
DG = HPG * HD          # 256 dims per group
SCALE = 8.0
NCORES = 8
NQC = S // 512         # 4 query chunks
NJ = S // 128          # 16 key tiles
KC = D // 128          # 8 contraction chunks
BF16 = ml_dtypes.bfloat16

_CACHE = {}


def _build(causal: bool):
    import concourse.mybir as mybir
    import concourse.tile as tile
    from concourse import bacc

    f32 = mybir.dt.float32
    b16 = mybir.dt.bfloat16
    Exp = mybir.ActivationFunctionType.Exp

    nc = bacc.Bacc(None, target_bir_lowering=False)

    qT = nc.dram_tensor("qT", [D, S], b16, kind="ExternalInput")
    kT = nc.dram_tensor("kT", [D, S], b16, kind="ExternalInput")
    vT = nc.dram_tensor("vT", [D, S], b16, kind="ExternalInput")
    # weights host-prepacked to the exact SBUF tile layout (one DMA each)
    # wq/wk: m-major [128, m*D + kc*128] so the m=0 projection can start
    # after only the first half of the weight lands
    wqT = nc.dram_tensor("wqT", [128, 2 * D], b16, kind="ExternalInput")
    wkT = nc.dram_tensor("wkT", [128, 2 * D], b16, kind="ExternalInput")
    wvT = nc.dram_tensor("wvT", [128, KC * DG], b16, kind="ExternalInput")
    woT = nc.dram_tensor("woT", [128, 2 * D], b16, kind="ExternalInput")
    bq = nc.dram_tensor("bq", [128, 2], f32, kind="ExternalInput")
    bk = nc.dram_tensor("bk", [128, 2], f32, kind="ExternalInput")
    bvb = nc.dram_tensor("bvb", [128, DG], b16, kind="ExternalInput")
    tri = nc.dram_tensor("tri", [128, 128], b16, kind="ExternalInput")
    out = nc.dram_tensor("out", [D, S], b16, kind="ExternalOutput")

    with tile.TileContext(nc) as tc:
        with (
            tc.tile_pool(name="consts", bufs=1) as consts,
            tc.tile_pool(name="proj", bufs=1) as proj,
            tc.tile_pool(name="pin", bufs=1) as pin,
            tc.tile_pool(name="probs", bufs=8) as probsp,
            tc.tile_pool(name="rec", bufs=2) as recp,
            tc.tile_pool(name="ost", bufs=3) as ostp,
            tc.tile_pool(name="mm", bufs=3, space="PSUM") as mmp,
            tc.tile_pool(name="cu", bufs=1, space="PSUM") as cup,
        ):
            # --- constant tiles -------------------------------------------
            wq_t = consts.tile([128, 2 * D], b16)
            wk_t = consts.tile([128, 2 * D], b16)
            wv_t = consts.tile([128, KC * DG], b16)
            wo_t = consts.tile([128, 2 * D], b16)
            bq_t = consts.tile([128, 2], f32)
            bk_t = consts.tile([128, 2], f32)
            bvb_t = consts.tile([128, DG], b16)
            tri_t = consts.tile([128, 128], b16)
            warm_sb = consts.tile([128, 128], b16)
            nc.vector.memset(warm_sb[:], 0.0)

            # --- persistent projection outputs ----------------------------
            # qpT/kpT: pair p in cols [p*S,(p+1)*S); rows 0:64 head 2p, 64:128 head 2p+1
            qpT = proj.tile([128, 2 * S], b16)
            kpT = proj.tile([128, 2 * S], b16)
            # vp: key tile j at cols [j*512,(j+1)*512); head h at +h*128:
            # cols 0:64 = projected v, cols 64:128 = 1.0 (sumexp column block)
            vp = proj.tile([128, NJ * 512], b16)
            nc.vector.memset(vp[:], 1.0)
            # ctxT: same pair layout as qpT, normalized attention output (c x s)
            ctxT = proj.tile([128, 2 * S], b16)

            # --- input row tiles + DMA schedule ---------------------------
            # quarter-split column DMAs so compute starts as soon as the
            # first 512 columns of each row land
            qrow = [pin.tile([128, S], b16, name=f"qrow{kc}") for kc in range(KC)]
            krow = [pin.tile([128, S], b16, name=f"krow{kc}") for kc in range(KC)]
            vrow = [pin.tile([128, S], b16, name=f"vrow{kc}") for kc in range(KC)]

            def dma_rows(rows, src, cs):
                for kc in range(KC):
                    nc.sync.dma_start(rows[kc][:, cs], src[kc * 128:(kc + 1) * 128, cs])

            def dma_w4(w_t, src_w):
                # 4 parallel chunks so the weight lands ~4x sooner
                for ch in range(4):
                    cs = slice(ch * 512, (ch + 1) * 512)
                    nc.gpsimd.dma_start(w_t[:, cs], src_w[:, cs])

            q0, q1, half1 = slice(0, 512), slice(512, 1024), slice(1024, 2048)
            # weights on the gpsimd DMA queue, input rows on the sync queue:
            # two descriptor issuers in parallel so first bytes land sooner.
            # chunk 0 needs only the first 512 columns of q/k/v, so those
            # stream first.
            for ch in range(4):
                cs = slice(ch * 512, (ch + 1) * 512)
                nc.gpsimd.dma_start(wq_t[:, cs], wqT[:, cs])
            dma_rows(qrow, qT, q0)
            for ch in range(4):
                cs = slice(ch * 512, (ch + 1) * 512)
                nc.gpsimd.dma_start(wk_t[:, cs], wkT[:, cs])
            nc.gpsimd.dma_start(bq_t[:], bq[:])
            nc.gpsimd.dma_start(bk_t[:], bk[:])
            nc.gpsimd.dma_start(bvb_t[:], bvb[:])
            nc.gpsimd.dma_start(tri_t[:], tri[:])
            dma_rows(krow, kT, q0)
            dma_w4(wv_t, wvT)
            dma_rows(vrow, vT, q0)
            dma_w4(wo_t, woT)
            dma_rows(qrow, qT, q1)
            dma_rows(krow, kT, q1)
            dma_rows(vrow, vT, q1)
            dma_rows(qrow, qT, half1)
            dma_rows(krow, kT, half1)
            dma_rows(vrow, vT, half1)

            # warmup burst: keeps the PE activity monitor at full clock
            # while the first input quarters stream in
            warm_ps = mmp.tile([128, 1024], f32, tag="sc", name="warm")
            for wi in range(48):
                nc.tensor.matmul(warm_ps[:, 0:128], warm_sb[:], warm_sb[:],
                                 start=(wi == 0), stop=(wi == 47))
            # preload the exp spline tables (~2.7us) during the DMA window
            nc.scalar.activation(warm_sb[:, 0:1], warm_sb[:, 0:1], Exp)

            # --- projections interleaved with attention, per quarter ------
            # (single psum pool assigns slots in emission order, so program
            # order must follow the dataflow for cross-phase overlap)

            def qk_proj_m(name, rows, w_t, dst, bias_t, n, m):
                ps = mmp.tile([128, 1024], f32, tag="sc", name=f"{name}ps{m}{n}")[:, 0:512]
                for kc in range(KC):
                    nc.tensor.matmul(
                        ps,
                        w_t[:, m * D + kc * 128: m * D + (kc + 1) * 128],
                        rows[kc][:, n * 512:(n + 1) * 512],
                        start=(kc == 0), stop=(kc == KC - 1),
                    )
                nc.vector.tensor_scalar_add(
                    dst[:, m * S + n * 512: m * S + (n + 1) * 512],
                    ps, bias_t[:, m:m + 1],
                )

            def qk_proj(name, rows, w_t, dst, bias_t, n):
                for m in range(2):
                    qk_proj_m(name, rows, w_t, dst, bias_t, n, m)

            def v_proj_j(j):
                for j in [j]:
                    psf = mmp.tile([128, 1024], f32, tag="sc", name=f"vps{j}")
                    ps = psf[:, 0:DG]
                    for kc in range(KC):
                        nc.tensor.matmul(
                            ps, vrow[kc][:, j * 128:(j + 1) * 128],
                            wv_t[:, kc * DG:(kc + 1) * DG],
                            start=(kc == 0), stop=(kc == KC - 1),
                        )
                    # copy into the per-head 128-wide slots, adding the v
                    # bias: even heads [vp | ones] (ctx rows 0:64), odd heads
                    # [ones | vp] (ctx rows 64:128) so each head's ctx lands
                    # on the ctxT lanes it must be written to.  One strided
                    # op per parity (heads h and h+2 share offsets mod 256).
                    ps_r = psf.rearrange("p (t y) -> p t y", y=128)
                    vp_r = vp.rearrange("p (t y) -> p t y", y=256)
                    bv_r = bvb_t.rearrange("p (t y) -> p t y", y=128)
                    nc.vector.tensor_add(
                        vp_r[:, 2 * j:2 * j + 2, 0:HD],
                        ps_r[:, 0:2, 0:HD], bv_r[:, 0:2, 0:HD])
                    nc.vector.tensor_add(
                        vp_r[:, 2 * j:2 * j + 2, 128 + HD:256],
                        ps_r[:, 0:2, HD:128], bv_r[:, 0:2, HD:128])

            def v_proj(n):
                for j in range(4 * n, 4 * n + 4):
                    v_proj_j(j)

            def attn_j_sc(c, p, j, nj):
                qoff = p * S + c * 512
                d = j - 4 * c if causal else -1
                coff = 0 if d < 0 else 128 * d
                sc = mmp.tile([128, 1024], f32, tag="sc", name=f"sc{c}{p}{j}")
                for hh, (rlo, rhi) in enumerate(((0, 64), (64, 128))):
                    nc.tensor.matmul(
                        sc[:, hh * 512 + coff: hh * 512 + 512],
                        kpT[rlo:rhi, p * S + j * 128: p * S + (j + 1) * 128],
                        qpT[rlo:rhi, qoff + coff: qoff + 512],
                        start=True, stop=True, tile_position=(rlo, 0),
                    )
                pr = probsp.tile([128, 1024], b16, tag="pr", name=f"pr{c}{p}{j}")
                if coff == 0:
                    nc.scalar.activation(pr[:, 0:1024], sc[:, 0:1024], Exp, scale=1.0 / SCALE)
                else:
                    sc_v = sc.rearrange("p (h n) -> p h n", h=2)[:, :, coff:512]
                    pr_v = pr.rearrange("p (h n) -> p h n", h=2)[:, :, coff:512]
                    nc.scalar.activation(pr_v, sc_v, Exp, scale=1.0 / SCALE)
                if d >= 0:
                    for hh in range(2):
                        band = pr[:, hh * 512 + coff: hh * 512 + coff + 128]
                        nc.vector.tensor_mul(band, band, tri_t[:])
                return pr

            def attn_j_pv(c, p, j, nj, hps, pr):
                d = j - 4 * c if causal else -1
                coff = 0 if d < 0 else 128 * d
                first, last = (j == 0), (j == nj - 1)
                for hh in range(2):
                    h = 2 * p + hh
                    nc.tensor.matmul(
                        hps[:, hh * 512 + coff: hh * 512 + 512],
                        vp[:, j * 512 + h * 128: j * 512 + (h + 1) * 128],
                        pr[:, hh * 512 + coff: hh * 512 + 512],
                        start=first, stop=last, skip_group_check=True,
                    )

            def attn_pair(c, p, nj, hps, bg=None, every=2, lag=3):
                # bg: list of zero-arg emitters (projection groups) woven
                # between attention slots so their psum-slot turns come up
                # mid-stream and their matmuls fill PE slack under the
                # ACT-paced softmax.
                # lag: PV trails the scores by `lag` key tiles so the first
                # PV (which waits for the previous pair's norm to release
                # the shared ctx psum) enters the in-order PE queue late
                # enough not to stall it.
                bg = list(bg or [])
                prs = {}
                for j in range(nj):
                    prs[j] = attn_j_sc(c, p, j, nj)
                    if j >= lag:
                        attn_j_pv(c, p, j - lag, nj, hps, prs.pop(j - lag))
                    if bg and (j + 1) % every == 0:
                        bg.pop(0)()
                # leftover bg first: the trailing PVs may consume v
                # projections still sitting in the bg list
                while bg:
                    bg.pop(0)()
                for j in sorted(prs):
                    attn_j_pv(c, p, j, nj, hps, prs.pop(j))

            def norm_pair(c, p, hps):
                # hps cols 0:512 = even head (ctx rows 0:64, sum rows
                # 64:128); cols 512:1024 = odd head (sum rows 0:64, ctx rows
                # 64:128).  DVE lanes are independent pipelines, so the
                # reciprocal runs on the sum's own lanes and a tiny
                # SBUF->SBUF DMA moves it across lanes to line up with ctx.
                # reciprocal_approx_fast only works at partition offset 0,
                # and GpSimd (the only cross-lane engine) has no PSUM port:
                # stage sums to SBUF lane-aligned, cross-lane via gpsimd
                # copy, reciprocal at offset 0.
                st = recp.tile([128, 1024], f32, tag="st", name=f"st{c}{p}")
                ra = recp.tile([128, 1024], f32, tag="ra", name=f"ra{c}{p}")
                nc.vector.reciprocal_approx_fast(st[0:64, 512:1024], hps[0:64, 512:1024])
                nc.vector.tensor_copy(st[64:128, 0:512], hps[64:128, 0:512])
                nc.gpsimd.dma_start(ra[0:64, 0:512], st[64:128, 0:512])
                nc.gpsimd.dma_start(ra[64:128, 512:1024], st[0:64, 512:1024])
                nc.vector.reciprocal_approx_fast(ra[0:64, 0:512], ra[0:64, 0:512])
                nc.vector.tensor_mul(
                    ctxT[0:64, p * S + c * 512: p * S + (c + 1) * 512],
                    hps[0:64, 0:512], ra[0:64, 0:512])
                nc.vector.tensor_mul(
                    ctxT[64:128, p * S + c * 512: p * S + (c + 1) * 512],
                    hps[64:128, 512:1024], ra[64:128, 512:1024])

            def oproj_dc(c, dc):
                for dc in [dc]:
                    ops = mmp.tile([128, 1024], f32, tag="sc", name=f"op{c}{dc}")[:, 0:512]
                    for p2 in range(2):
                        nc.tensor.matmul(
                            ops,
                            wo_t[:, p2 * D + dc * 128: p2 * D + (dc + 1) * 128],
                            ctxT[:, p2 * S + c * 512: p2 * S + (c + 1) * 512],
                            start=(p2 == 0), stop=(p2 == 1),
                        )
                    ot = ostp.tile([128, 512], b16, tag="ot", name=f"ot{c}{dc}")
                    nc.vector.tensor_copy(ot[:], ops)
                    nc.sync.dma_start(
                        out[dc * 128:(dc + 1) * 128, c * 512:(c + 1) * 512], ot[:])

            def oproj(c, dcs=range(KC)):
                for dc in dcs:
                    oproj_dc(c, dc)

            qk_proj("q", qrow, wq_t, qpT, bq_t, 0)
            if not causal:
                # no diagonal structure to pipeline against: project all
                # quarters upfront
                for n in range(NQC):
                    if n > 0:
                        qk_proj("q", qrow, wq_t, qpT, bq_t, n)
                    qk_proj("k", krow, wk_t, kpT, bk_t, n)
                    v_proj(n)
            for c in range(NQC):
                nj = 4 * c + 4 if causal else NJ
                hps0 = cup.tile([128, 1024], f32, tag="cu", name=f"cu{c}0")
                # part A (earlier-quarter key tiles; needs only qpT of this
                # quarter) with this quarter's k/v projections woven between
                # slots so they retire before the diagonal part B needs them
                bg_a = []
                if causal:
                    bg_a += [lambda m=m: qk_proj_m("k", krow, wk_t, kpT, bk_t, c, m) for m in range(2)]
                    bg_a += [lambda j=j: v_proj_j(j) for j in range(4 * c, 4 * c + 4)]
                if c == 0:
                    # no pre-diagonal score tiles to weave against: the k/v
                    # projections must land before the first scores read them
                    while bg_a:
                        bg_a.pop(0)()
                attn_pair(c, 0, nj, hps0, bg=bg_a, every=1)
                norm_pair(c, 0, hps0)
                # pair 1 hosts: previous chunk's output projection and the
                # next quarter's q projection
                bg_b = []
                if c > 0:
                    bg_b += [lambda dc=dc: oproj_dc(c - 1, dc) for dc in range(KC)]
                if causal and c + 1 < NQC:
                    bg_b += [lambda m=m: qk_proj_m("q", qrow, wq_t, qpT, bq_t, c + 1, m) for m in range(2)]
                hps1 = cup.tile([128, 1024], f32, tag="cu", name=f"cu{c}1")
                attn_pair(c, 1, nj, hps1, bg=bg_b, every=2)
                norm_pair(c, 1, hps1)
            oproj(NQC - 1)

    nc.compile()
    return nc


def _get_nc(causal: bool):
    if causal not in _CACHE:
        _CACHE[causal] = _build(causal)
    return _CACHE[causal]


def _pack_w(w):
    # [D, DG] -> SBUF layout [128, KC*DG]: chunk kc of 128 rows side by side
    return np.ascontiguousarray(w.reshape(KC, 128, DG).transpose(1, 0, 2).reshape(128, KC * DG)).astype(BF16)


def _pack_w_mmajor(w):
    # [D, DG] -> SBUF layout [128, 2*D]: half m of the 256 out dims at
    # cols m*D, within that chunk kc at +kc*128
    return np.ascontiguousarray(
        w.reshape(KC, 128, 2, 128).transpose(1, 2, 0, 3).reshape(128, 2 * D)).astype(BF16)


def make_in_maps(q, k, v, w_q, b_q, w_k, b_k, w_v, b_v, w_o):
    tri_keep = (np.arange(128)[:, None] <= np.arange(128)[None, :]).astype(BF16)
    qT = [np.ascontiguousarray(q[b].T).astype(BF16) for b in range(B)]
    kTn = [np.ascontiguousarray(k[b].T).astype(BF16) for b in range(B)]
    vTn = [np.ascontiguousarray(v[b].T).astype(BF16) for b in range(B)]
    in_maps = []
    for core in range(NCORES):
        b, g = core // G, core % G
        sl = slice(g * DG, (g + 1) * DG)
        woTg = np.ascontiguousarray(w_o[:, sl].T)  # [DG, D]
        in_maps.append({
            "qT": qT[b], "kT": kTn[b], "vT": vTn[b],
            "wqT": _pack_w_mmajor(np.ascontiguousarray(w_q[sl, :].T)),
            "wkT": _pack_w_mmajor(np.ascontiguousarray(w_k[sl, :].T)),
            "wvT": _pack_w(np.ascontiguousarray(w_v[sl, :].T)),
            "woT": np.ascontiguousarray(
                woTg.reshape(2, 128, D).transpose(1, 0, 2).reshape(128, 2 * D)).astype(BF16),
            "bq": np.ascontiguousarray(b_q[sl].reshape(2, 128).T).astype(np.float32),
            "bk": np.ascontiguousarray(b_k[sl].reshape(2, 128).T).astype(np.float32),
            "bvb": np.ascontiguousarray(np.broadcast_to(b_v[sl], (128, DG))).astype(BF16),
            "tri": tri_keep,
        })
    return in_maps


def _reference_numpy(q, k, v, mask, w_q, b_q, w_k, b_k, w_v, b_v, w_o, b_o):
    qp = q @ w_q.T + b_q
    kp = k @ w_k.T + b_k
    vv = v @ w_v.T + b_v
    qp = qp.reshape(B, S, H, HD).transpose(0, 2, 1, 3)
    kp = kp.reshape(B, S, H, HD).transpose(0, 2, 1, 3)
    vv = vv.reshape(B, S, H, HD).transpose(0, 2, 1, 3)
    score = np.einsum("bhqd,bhkd->bhqk", qp, kp) / SCALE
    score = np.where(mask, -1e9, score)
    score -= score.max(axis=-1, keepdims=True)
    e = np.exp(score)
    attn = e / e.sum(axis=-1, keepdims=True)
    ctx = np.einsum("bhqk,bhkd->bhqd", attn, vv)
    ctx = ctx.transpose(0, 2, 1, 3).reshape(B, S, D)
    return (ctx @ w_o.T + b_o).astype(np.float32)


def kernel(q, k, v, mask, w_q, b_q, w_k, b_k, w_v, b_v, w_o, b_o):
    from concourse.bass_utils import run_bass_kernel_spmd

    q, k, v = (np.asarray(x, np.float32) for x in (q, k, v))
    mask = np.asarray(mask)
    causal_ref = np.triu(np.ones((S, S), bool), k=1)
    causal = all(np.array_equal(mask[b, 0], causal_ref) for b in range(B))
    if not causal and mask.any():
        # Unexpected mask pattern: fall back to exact numpy (never hit in
        # practice -- setup_inputs always builds the causal mask).
        return _reference_numpy(q, k, v, mask, w_q, b_q, w_k, b_k, w_v, b_v, w_o, b_o)

    nc = _get_nc(causal)
    in_maps = make_in_maps(q, k, v, w_q, b_q, w_k, b_k, w_v, b_v, w_o)
    res = run_bass_kernel_spmd(nc, in_maps, core_ids=list(range(NCORES)))

    out = np.zeros((B, S, D), np.float32)
    for core in range(NCORES):
        b = core // G
        out[b] += res.results[core]["out"].T.astype(np.float32)
    out += np.asarray(b_o, np.float32)
    return out


# revision 20
# speedup vs baseline: 1.0509x; 1.0078x over previous
"""Multi-head attention (B=2,S=2048,D=1024,H=16) on 8 trn2 NeuronCores.

Sharding: core = b*4 + g  (b = batch 0..1, g = head-group 0..3, 4 heads each).
Each core computes QKV projections for its 256 output dims, causal attention
for its 4 heads (scores kept transposed: [s_k, s_q]), and a K-sliced partial
of the output projection (transposed: [D, S]).  Host sums the 4 partials per
batch and adds b_o.

All matmuls in bf16 (fp32 PSUM accumulate); softmax without max-subtraction
(scores/8 are small, exp cannot overflow).  The PV stationary for head h is
[vp_h | ones] (128 wide): out rows 0:64 accumulate P@V, rows 64:128
accumulate sum(P) broadcast 64-wide -- sumexp costs zero extra PE streams.
Normalization is reciprocal + elementwise multiply on DVE.  v-bias is added
by the DVE copy that moves the v-projection psum into the [vp|ones] layout.
"""
import sys

if "/opt/trn_rl_repo" not in sys.path:
    sys.path.insert(0, "/opt/trn_rl_repo")

import numpy as np
import ml_dtypes

B, S, D, H = 2, 2048, 1024, 16
HD = D // H            # 64
G = 4                  # head groups (one per core within a batch)
HPG = H // G           # 4 heads per group
DG = HPG * HD          # 256 dims per group
SCALE = 8.0
NCORES = 8
NQC = S // 512         # 4 query chunks
NJ = S // 128          # 16 key tiles
KC = D // 128          # 8 contraction chunks
BF16 = ml_dtypes.bfloat16

_CACHE = {}


def _build(causal: bool):
    import concourse.mybir as mybir
    import concourse.tile as tile
    from concourse import bacc

    f32 = mybir.dt.float32
    b16 = mybir.dt.bfloat16
    Exp = mybir.ActivationFunctionType.Exp

    nc = bacc.Bacc(None, target_bir_lowering=False)

    qT = nc.dram_tensor("qT", [D, S], b16, kind="ExternalInput")
    kT = nc.dram_tensor("kT", [D, S], b16, kind="ExternalInput")
    vT = nc.dram_tensor("vT", [D, S], b16, kind="ExternalInput")
    # weights host-prepacked to the exact SBUF tile layout (one DMA each)
    # wq/wk: m-major [128, m*D + kc*128] so the m=0 projection can start
    # after only the first half of the weight lands
    wqT = nc.dram_tensor("wqT", [128, 2 * D], b16, kind="ExternalInput")
    wkT = nc.dram_tensor("wkT", [128, 2 * D], b16, kind="ExternalInput")
    wvT = nc.dram_tensor("wvT", [128, KC * DG], b16, kind="ExternalInput")
    woT = nc.dram_tensor("woT", [128, 2 * D], b16, kind="ExternalInput")
    bq = nc.dram_tensor("bq", [128, 2], f32, kind="ExternalInput")
    bk = nc.dram_tensor("bk", [128, 2], f32, kind="ExternalInput")
    bvb = nc.dram_tensor("bvb", [128, DG], b16, kind="ExternalInput")
    tri = nc.dram_tensor("tri", [128, 128], b16, kind="ExternalInput")
    swp = nc.dram_tensor("swp", [128, 128], b16, kind="ExternalInput")
    out = nc.dram_tensor("out", [D, S], b16, kind="ExternalOutput")

    with tile.TileContext(nc) as tc:
        with (
            tc.tile_pool(name="consts", bufs=1) as consts,
            tc.tile_pool(name="proj", bufs=1) as proj,
            tc.tile_pool(name="pin", bufs=1) as pin,
            tc.tile_pool(name="probs", bufs=8) as probsp,
            tc.tile_pool(name="rec", bufs=2) as recp,
            tc.tile_pool(name="ost", bufs=3) as ostp,
            tc.tile_pool(name="mm", bufs=3, space="PSUM") as mmp,
            tc.tile_pool(name="cu", bufs=1, space="PSUM") as cup,
        ):
            # --- constant tiles -------------------------------------------
            wq_t = consts.tile([128, 2 * D], b16)
            wk_t = consts.tile([128, 2 * D], b16)
            wv_t = consts.tile([128, KC * DG], b16)
            wo_t = consts.tile([128, 2 * D], b16)
            bq_t = consts.tile([128, 2], f32)
            bk_t = consts.tile([128, 2], f32)
            bvb_t = consts.tile([128, DG], b16)
            tri_t = consts.tile([128, 128], b16)
            swp_t = consts.tile([128, 128], b16)
            warm_sb = consts.tile([128, 128], b16)
            nc.vector.memset(warm_sb[:], 0.0)

            # --- persistent projection outputs ----------------------------
            # qpT/kpT: pair p in cols [p*S,(p+1)*S); rows 0:64 head 2p, 64:128 head 2p+1
            qpT = proj.tile([128, 2 * S], b16)
            kpT = proj.tile([128, 2 * S], b16)
            # vp: key tile j at cols [j*512,(j+1)*512); head h at +h*128:
            # cols 0:64 = projected v, cols 64:128 = 1.0 (sumexp column block)
            vp = proj.tile([128, NJ * 512], b16)
            nc.vector.memset(vp[:], 1.0)
            # ctxT: same pair layout as qpT, normalized attention output (c x s)
            ctxT = proj.tile([128, 2 * S], b16)

            # --- input row tiles + DMA schedule ---------------------------
            # quarter-split column DMAs so compute starts as soon as the
            # first 512 columns of each row land
            qrow = [pin.tile([128, S], b16, name=f"qrow{kc}") for kc in range(KC)]
            krow = [pin.tile([128, S], b16, name=f"krow{kc}") for kc in range(KC)]
            vrow = [pin.tile([128, S], b16, name=f"vrow{kc}") for kc in range(KC)]

            def dma_rows(rows, src, cs):
                for kc in range(KC):
                    nc.sync.dma_start(rows[kc][:, cs], src[kc * 128:(kc + 1) * 128, cs])

            def dma_w4(w_t, src_w):
                # 4 parallel chunks so the weight lands ~4x sooner
                for ch in range(4):
                    cs = slice(ch * 512, (ch + 1) * 512)
                    nc.gpsimd.dma_start(w_t[:, cs], src_w[:, cs])

            q0, q1, half1 = slice(0, 512), slice(512, 1024), slice(1024, 2048)
            # weights on the gpsimd DMA queue, input rows on the sync queue:
            # two descriptor issuers in parallel so first bytes land sooner.
            # chunk 0 needs only the first 512 columns of q/k/v, so those
            # stream first.
            for ch in range(4):
                cs = slice(ch * 512, (ch + 1) * 512)
                nc.gpsimd.dma_start(wq_t[:, cs], wqT[:, cs])
            dma_rows(qrow, qT, q0)
            for ch in range(4):
                cs = slice(ch * 512, (ch + 1) * 512)
                nc.gpsimd.dma_start(wk_t[:, cs], wkT[:, cs])
            nc.gpsimd.dma_start(bq_t[:], bq[:])
            nc.gpsimd.dma_start(bk_t[:], bk[:])
            nc.gpsimd.dma_start(bvb_t[:], bvb[:])
            nc.gpsimd.dma_start(tri_t[:], tri[:])
            nc.gpsimd.dma_start(swp_t[:], swp[:])
            dma_rows(krow, kT, q0)
            dma_w4(wv_t, wvT)
            dma_rows(vrow, vT, q0)
            dma_w4(wo_t, woT)
            dma_rows(qrow, qT, q1)
            dma_rows(krow, kT, q1)
            dma_rows(vrow, vT, q1)
            dma_rows(qrow, qT, half1)
            dma_rows(krow, kT, half1)
            dma_rows(vrow, vT, half1)

            # warmup burst: keeps the PE activity monitor at full clock
            # while the first input quarters stream in
            warm_ps = mmp.tile([128, 1024], f32, tag="sc", name="warm")
            for wi in range(48):
                nc.tensor.matmul(warm_ps[:, 0:128], warm_sb[:], warm_sb[:],
                                 start=(wi == 0), stop=(wi == 47))
            # preload the exp spline tables (~2.7us) during the DMA window
            nc.scalar.activation(warm_sb[:, 0:1], warm_sb[:, 0:1], Exp)

            # --- projections interleaved with attention, per quarter ------
            # (single psum pool assigns slots in emission order, so program
            # order must follow the dataflow for cross-phase overlap)

            def qk_proj_m(name, rows, w_t, dst, bias_t, n, m):
                ps = mmp.tile([128, 1024], f32, tag="sc", name=f"{name}ps{m}{n}")[:, 0:512]
                for kc in range(KC):
                    nc.tensor.matmul(
                        ps,
                        w_t[:, m * D + kc * 128: m * D + (kc + 1) * 128],
                        rows[kc][:, n * 512:(n + 1) * 512],
                        start=(kc == 0), stop=(kc == KC - 1),
                    )
                nc.vector.tensor_scalar_add(
                    dst[:, m * S + n * 512: m * S + (n + 1) * 512],
                    ps, bias_t[:, m:m + 1],
                )

            def qk_proj(name, rows, w_t, dst, bias_t, n):
                for m in range(2):
                    qk_proj_m(name, rows, w_t, dst, bias_t, n, m)

            def v_proj_j(j):
                for j in [j]:
                    psf = mmp.tile([128, 1024], f32, tag="sc", name=f"vps{j}")
                    ps = psf[:, 0:DG]
                    for kc in range(KC):
                        nc.tensor.matmul(
                            ps, vrow[kc][:, j * 128:(j + 1) * 128],
                            wv_t[:, kc * DG:(kc + 1) * DG],
                            start=(kc == 0), stop=(kc == KC - 1),
                        )
                    # copy into the per-head 128-wide slots, adding the v
                    # bias: even heads [vp | ones] (ctx rows 0:64), odd heads
                    # [ones | vp] (ctx rows 64:128) so each head's ctx lands
                    # on the ctxT lanes it must be written to.  One strided
                    # op per parity (heads h and h+2 share offsets mod 256).
                    ps_r = psf.rearrange("p (t y) -> p t y", y=128)
                    vp_r = vp.rearrange("p (t y) -> p t y", y=256)
                    bv_r = bvb_t.rearrange("p (t y) -> p t y", y=128)
                    nc.vector.tensor_add(
                        vp_r[:, 2 * j:2 * j + 2, 0:HD],
                        ps_r[:, 0:2, 0:HD], bv_r[:, 0:2, 0:HD])
                    nc.vector.tensor_add(
                        vp_r[:, 2 * j:2 * j + 2, 128 + HD:256],
                        ps_r[:, 0:2, HD:128], bv_r[:, 0:2, HD:128])

            def v_proj(n):
                for j in range(4 * n, 4 * n + 4):
                    v_proj_j(j)

            def attn_j_sc(c, p, j, nj):
                qoff = p * S + c * 512
                d = j - 4 * c if causal else -1
                coff = 0 if d < 0 else 128 * d
                sc = mmp.tile([128, 1024], f32, tag="sc", name=f"sc{c}{p}{j}")
                for hh, (rlo, rhi) in enumerate(((0, 64), (64, 128))):
                    nc.tensor.matmul(
                        sc[:, hh * 512 + coff: hh * 512 + 512],
                        kpT[rlo:rhi, p * S + j * 128: p * S + (j + 1) * 128],
                        qpT[rlo:rhi, qoff + coff: qoff + 512],
                        start=True, stop=True, tile_position=(rlo, 0),
                    )
                pr = probsp.tile([128, 1024], b16, tag="pr", name=f"pr{c}{p}{j}")
                if coff == 0:
                    nc.scalar.activation(pr[:, 0:1024], sc[:, 0:1024], Exp, scale=1.0 / SCALE)
                else:
                    sc_v = sc.rearrange("p (h n) -> p h n", h=2)[:, :, coff:512]
                    pr_v = pr.rearrange("p (h n) -> p h n", h=2)[:, :, coff:512]
                    nc.scalar.activation(pr_v, sc_v, Exp, scale=1.0 / SCALE)
                if d >= 0:
                    for hh in range(2):
                        band = pr[:, hh * 512 + coff: hh * 512 + coff + 128]
                        nc.vector.tensor_mul(band, band, tri_t[:])
                return pr

            def attn_j_pv(c, p, j, nj, hps, pr):
                d = j - 4 * c if causal else -1
                coff = 0 if d < 0 else 128 * d
                first, last = (j == 0), (j == nj - 1)
                for hh in range(2):
                    h = 2 * p + hh
                    nc.tensor.matmul(
                        hps[:, hh * 512 + coff: hh * 512 + 512],
                        vp[:, j * 512 + h * 128: j * 512 + (h + 1) * 128],
                        pr[:, hh * 512 + coff: hh * 512 + 512],
                        start=first, stop=last, skip_group_check=True,
                    )

            def attn_pair(c, p, nj, hps, bg=None, every=2, lag=3):
                # bg: list of zero-arg emitters (projection groups) woven
                # between attention slots so their psum-slot turns come up
                # mid-stream and their matmuls fill PE slack under the
                # ACT-paced softmax.
                # lag: PV trails the scores by `lag` key tiles so the first
                # PV (which waits for the previous pair's norm to release
                # the shared ctx psum) enters the in-order PE queue late
                # enough not to stall it.
                bg = list(bg or [])
                prs = {}
                for j in range(nj):
                    prs[j] = attn_j_sc(c, p, j, nj)
                    if j >= lag:
                        attn_j_pv(c, p, j - lag, nj, hps, prs.pop(j - lag))
                    if bg and (j + 1) % every == 0:
                        bg.pop(0)()
                # leftover bg first: the trailing PVs may consume v
                # projections still sitting in the bg list
                while bg:
                    bg.pop(0)()
                for j in sorted(prs):
                    attn_j_pv(c, p, j, nj, hps, prs.pop(j))

            def norm_pair(c, p, hps):
                # hps cols 0:512 = even head (ctx rows 0:64, sum rows
                # 64:128); cols 512:1024 = odd head (sum rows 0:64, ctx rows
                # 64:128).  DVE lanes are independent pipelines, so the
                # reciprocal runs on the sum's own lanes and a tiny
                # SBUF->SBUF DMA moves it across lanes to line up with ctx.
                # Evacuate the whole pair psum to SBUF (frees the single cu
                # psum buffer ~1.2us after the last PV), then swap lanes
                # 0:64 <-> 64:128 with a constant permutation matmul so each
                # head's sum lands on its ctx lanes, reciprocal at offset 0,
                # lane-aligned multiplies.  (DVE lanes can't cross; the PE
                # swap costs ~0.4us.)
                st = recp.tile([128, 1024], b16, tag="st", name=f"st{c}{p}")
                nc.vector.tensor_copy(st[:], hps[:])
                sw = mmp.tile([128, 1024], f32, tag="sc", name=f"sw{c}{p}")
                for hh in range(2):
                    nc.tensor.matmul(sw[:, hh * 512:(hh + 1) * 512], swp_t[:],
                                     st[:, hh * 512:(hh + 1) * 512],
                                     start=True, stop=True)
                ra = recp.tile([128, 1024], f32, tag="ra", name=f"ra{c}{p}")
                nc.vector.reciprocal_approx_fast(ra[:], sw[:])
                nc.vector.tensor_mul(
                    ctxT[0:64, p * S + c * 512: p * S + (c + 1) * 512],
                    st[0:64, 0:512], ra[0:64, 0:512])
                nc.vector.tensor_mul(
                    ctxT[64:128, p * S + c * 512: p * S + (c + 1) * 512],
                    st[64:128, 512:1024], ra[64:128, 512:1024])

            def oproj_dc(c, dc):
                for dc in [dc]:
                    ops = mmp.tile([128, 1024], f32, tag="sc", name=f"op{c}{dc}")[:, 0:512]
                    for p2 in range(2):
                        nc.tensor.matmul(
                            ops,
                            wo_t[:, p2 * D + dc * 128: p2 * D + (dc + 1) * 128],
                            ctxT[:, p2 * S + c * 512: p2 * S + (c + 1) * 512],
                            start=(p2 == 0), stop=(p2 == 1),
                        )
                    ot = ostp.tile([128, 512], b16, tag="ot", name=f"ot{c}{dc}")
                    nc.vector.tensor_copy(ot[:], ops)
                    nc.sync.dma_start(
                        out[dc * 128:(dc + 1) * 128, c * 512:(c + 1) * 512], ot[:])

            def oproj(c, dcs=range(KC)):
                for dc in dcs:
                    oproj_dc(c, dc)

            qk_proj("q", qrow, wq_t, qpT, bq_t, 0)
            if not causal:
                # no diagonal structure to pipeline against: project all
                # quarters upfront
                for n in range(NQC):
                    if n > 0:
                        qk_proj("q", qrow, wq_t, qpT, bq_t, n)
                    qk_proj("k", krow, wk_t, kpT, bk_t, n)
                    v_proj(n)
            for c in range(NQC):
                nj = 4 * c + 4 if causal else NJ
                hps0 = cup.tile([128, 1024], f32, tag="cu", name=f"cu{c}0")
                # part A (earlier-quarter key tiles; needs only qpT of this
                # quarter) with this quarter's k/v projections woven between
                # slots so they retire before the diagonal part B needs them
                bg_a = []
                if causal:
                    bg_a += [lambda m=m: qk_proj_m("k", krow, wk_t, kpT, bk_t, c, m) for m in range(2)]
                    bg_a += [lambda j=j: v_proj_j(j) for j in range(4 * c, 4 * c + 4)]
                if c == 0:
                    # no pre-diagonal score tiles to weave against: the k/v
                    # projections must land before the first scores read them
                    while bg_a:
                        bg_a.pop(0)()
                attn_pair(c, 0, nj, hps0, bg=bg_a, every=1)
                norm_pair(c, 0, hps0)
                # pair 1 hosts: previous chunk's output projection and the
                # next quarter's q projection
                bg_b = []
                if c > 0:
                    bg_b += [lambda dc=dc: oproj_dc(c - 1, dc) for dc in range(KC)]
                if causal and c + 1 < NQC:
                    bg_b += [lambda m=m: qk_proj_m("q", qrow, wq_t, qpT, bq_t, c + 1, m) for m in range(2)]
                hps1 = cup.tile([128, 1024], f32, tag="cu", name=f"cu{c}1")
                attn_pair(c, 1, nj, hps1, bg=bg_b, every=2)
                norm_pair(c, 1, hps1)
            oproj(NQC - 1)

    nc.compile()
    return nc


def _get_nc(causal: bool):
    if causal not in _CACHE:
        _CACHE[causal] = _build(causal)
    return _CACHE[causal]


def _pack_w(w):
    # [D, DG] -> SBUF layout [128, KC*DG]: chunk kc of 128 rows side by side
    return np.ascontiguousarray(w.reshape(KC, 128, DG).transpose(1, 0, 2).reshape(128, KC * DG)).astype(BF16)


def _pack_w_mmajor(w):
    # [D, DG] -> SBUF layout [128, 2*D]: half m of the 256 out dims at
    # cols m*D, within that chunk kc at +kc*128
    return np.ascontiguousarray(
        w.reshape(KC, 128, 2, 128).transpose(1, 2, 0, 3).reshape(128, 2 * D)).astype(BF16)


def make_in_maps(q, k, v, w_q, b_q, w_k, b_k, w_v, b_v, w_o):
    tri_keep = (np.arange(128)[:, None] <= np.arange(128)[None, :]).astype(BF16)
    # lane-swap permutation: out[i] = in[(i + 64) % 128]
    swap_m = np.zeros((128, 128), BF16)
    swap_m[(np.arange(128) + 64) % 128, np.arange(128)] = 1
    qT = [np.ascontiguousarray(q[b].T).astype(BF16) for b in range(B)]
    kTn = [np.ascontiguousarray(k[b].T).astype(BF16) for b in range(B)]
    vTn = [np.ascontiguousarray(v[b].T).astype(BF16) for b in range(B)]
    in_maps = []
    for core in range(NCORES):
        b, g = core // G, core % G
        sl = slice(g * DG, (g + 1) * DG)
        woTg = np.ascontiguousarray(w_o[:, sl].T)  # [DG, D]
        in_maps.append({
            "qT": qT[b], "kT": kTn[b], "vT": vTn[b],
            "wqT": _pack_w_mmajor(np.ascontiguousarray(w_q[sl, :].T)),
            "wkT": _pack_w_mmajor(np.ascontiguousarray(w_k[sl, :].T)),
            "wvT": _pack_w(np.ascontiguousarray(w_v[sl, :].T)),
            "woT": np.ascontiguousarray(
                woTg.reshape(2, 128, D).transpose(1, 0, 2).reshape(128, 2 * D)).astype(BF16),
            "bq": np.ascontiguousarray(b_q[sl].reshape(2, 128).T).astype(np.float32),
            "bk": np.ascontiguousarray(b_k[sl].reshape(2, 128).T).astype(np.float32),
            "bvb": np.ascontiguousarray(np.broadcast_to(b_v[sl], (128, DG))).astype(BF16),
            "tri": tri_keep,
            "swp": swap_m,
        })
    return in_maps


def _reference_numpy(q, k, v, mask, w_q, b_q, w_k, b_k, w_v, b_v, w_o, b_o):
    qp = q @ w_q.T + b_q
    kp = k @ w_k.T + b_k
    vv = v @ w_v.T + b_v
    qp = qp.reshape(B, S, H, HD).transpose(0, 2, 1, 3)
    kp = kp.reshape(B, S, H, HD).transpose(0, 2, 1, 3)
    vv = vv.reshape(B, S, H, HD).transpose(0, 2, 1, 3)
    score = np.einsum("bhqd,bhkd->bhqk", qp, kp) / SCALE
    score = np.where(mask, -1e9, score)
    score -= score.max(axis=-1, keepdims=True)
    e = np.exp(score)
    attn = e / e.sum(axis=-1, keepdims=True)
    ctx = np.einsum("bhqk,bhkd->bhqd", attn, vv)
    ctx = ctx.transpose(0, 2, 1, 3).reshape(B, S, D)
    return (ctx @ w_o.T + b_o).astype(np.float32)


def kernel(q, k, v, mask, w_q, b_q, w_k, b_k, w_v, b_v, w_o, b_o):
    from concourse.bass_utils import run_bass_kernel_spmd

    q, k, v = (np.asarray(x, np.float32) for x in (q, k, v))
    mask = np.asarray(mask)
    causal_ref = np.triu(np.ones((S, S), bool), k=1)
    causal = all(np.array_equal(mask[b, 0], causal_ref) for b in range(B))
    if not causal and mask.any():
        # Unexpected mask pattern: fall back to exact numpy (never hit in
        # practice -- setup_inputs always builds the causal mask).
        return _reference_numpy(q, k, v, mask, w_q, b_q, w_k, b_k, w_v, b_v, w_o, b_o)

    nc = _get_nc(causal)
    in_maps = make_in_maps(q, k, v, w_q, b_q, w_k, b_k, w_v, b_v, w_o)
    res = run_bass_kernel_spmd(nc, in_maps, core_ids=list(range(NCORES)))

    out = np.zeros((B, S, D), np.float32)
    for core in range(NCORES):
        b = core // G
        out[b] += res.results[core]["out"].T.astype(np.float32)
    out += np.asarray(b_o, np.float32)
    return out


# revision 23
# speedup vs baseline: 1.0850x; 1.0324x over previous
"""Multi-head attention (B=2,S=2048,D=1024,H=16) on 8 trn2 NeuronCores.

Sharding: core = b*4 + g  (b = batch 0..1, g = head-group 0..3, 4 heads each).
Each core computes QKV projections for its 256 output dims, causal attention
for its 4 heads (scores kept transposed: [s_k, s_q]), and a K-sliced partial
of the output projection (transposed: [D, S]).  Host sums the 4 partials per
batch and adds b_o.

All matmuls in bf16 (fp32 PSUM accumulate); softmax without max-subtraction
(scores/8 are small, exp cannot overflow).  The PV stationary for head h is
[vp_h | ones] (128 wide): out rows 0:64 accumulate P@V, rows 64:128
accumulate sum(P) broadcast 64-wide -- sumexp costs zero extra PE streams.
Normalization is reciprocal + elementwise multiply on DVE.  v-bias is added
by the DVE copy that moves the v-projection psum into the [vp|ones] layout.
"""
import sys

if "/opt/trn_rl_repo" not in sys.path:
    sys.path.insert(0, "/opt/trn_rl_repo")

import numpy as np
import ml_dtypes

B, S, D, H = 2, 2048, 1024, 16
HD = D // H            # 64
G = 4                  # head groups (one per core within a batch)
HPG = H // G           # 4 heads per group
DG = HPG * HD          # 256 dims per group
SCALE = 8.0
NCORES = 8
NQC = S // 512         # 4 query chunks
NJ = S // 128          # 16 key tiles
KC = D // 128          # 8 contraction chunks
BF16 = ml_dtypes.bfloat16

_CACHE = {}


def _build(causal: bool):
    import concourse.mybir as mybir
    import concourse.tile as tile
    from concourse import bacc

    f32 = mybir.dt.float32
    b16 = mybir.dt.bfloat16
    Exp = mybir.ActivationFunctionType.Exp

    nc = bacc.Bacc(None, target_bir_lowering=False)

    qT = nc.dram_tensor("qT", [D, S], b16, kind="ExternalInput")
    kT = nc.dram_tensor("kT", [D, S], b16, kind="ExternalInput")
    vT = nc.dram_tensor("vT", [D, S], b16, kind="ExternalInput")
    # weights host-prepacked to the exact SBUF tile layout (one DMA each)
    # wq/wk: m-major [128, m*D + kc*128] so the m=0 projection can start
    # after only the first half of the weight lands
    wqT = nc.dram_tensor("wqT", [128, 2 * D], b16, kind="ExternalInput")
    wkT = nc.dram_tensor("wkT", [128, 2 * D], b16, kind="ExternalInput")
    wvT = nc.dram_tensor("wvT", [128, KC * DG], b16, kind="ExternalInput")
    woT = nc.dram_tensor("woT", [128, 2 * D], b16, kind="ExternalInput")
    bq = nc.dram_tensor("bq", [128, 2], f32, kind="ExternalInput")
    bk = nc.dram_tensor("bk", [128, 2], f32, kind="ExternalInput")
    bvb = nc.dram_tensor("bvb", [128, DG], b16, kind="ExternalInput")
    tri = nc.dram_tensor("tri", [128, 128], b16, kind="ExternalInput")
    swp = nc.dram_tensor("swp", [128, 128], b16, kind="ExternalInput")
    out = nc.dram_tensor("out", [D, S], b16, kind="ExternalOutput")

    with tile.TileContext(nc) as tc:
        with (
            tc.tile_pool(name="consts", bufs=1) as consts,
            tc.tile_pool(name="proj", bufs=1) as proj,
            tc.tile_pool(name="pin", bufs=1) as pin,
            tc.tile_pool(name="probs", bufs=8) as probsp,
            tc.tile_pool(name="rec", bufs=2) as recp,
            tc.tile_pool(name="ost", bufs=3) as ostp,
            tc.tile_pool(name="mm", bufs=3, space="PSUM") as mmp,
            tc.tile_pool(name="cu", bufs=1, space="PSUM") as cup,
        ):
            # --- constant tiles -------------------------------------------
            wq_t = consts.tile([128, 2 * D], b16)
            wk_t = consts.tile([128, 2 * D], b16)
            wv_t = consts.tile([128, KC * DG], b16)
            wo_t = consts.tile([128, 2 * D], b16)
            bq_t = consts.tile([128, 2], f32)
            bk_t = consts.tile([128, 2], f32)
            bvb_t = consts.tile([128, DG], b16)
            tri_t = consts.tile([128, 128], b16)
            swp_t = consts.tile([128, 128], b16)
            warm_sb = consts.tile([128, 128], b16)
            nc.vector.memset(warm_sb[:], 0.0)

            # --- persistent projection outputs ----------------------------
            # qpT/kpT: pair p in cols [p*S,(p+1)*S); rows 0:64 head 2p, 64:128 head 2p+1
            qpT = proj.tile([128, 2 * S], b16)
            kpT = proj.tile([128, 2 * S], b16)
            # vp: key tile j at cols [j*512,(j+1)*512); head h at +h*128:
            # 64 cols projected v, 64 cols 1.0 (sumexp block; order flips
            # with head parity).  Memset on the idle GpSimd engine keeps the
            # early DVE queue free for the warmup.
            vp = proj.tile([128, NJ * 512], b16)
            nc.gpsimd.memset(vp[:], 1.0)
            # ctxT: same pair layout as qpT, normalized attention output (c x s)
            ctxT = proj.tile([128, 2 * S], b16)

            # --- input row tiles + DMA schedule ---------------------------
            # quarter-split column DMAs so compute starts as soon as the
            # first 512 columns of each row land
            qrow = [pin.tile([128, S], b16, name=f"qrow{kc}") for kc in range(KC)]
            krow = [pin.tile([128, S], b16, name=f"krow{kc}") for kc in range(KC)]
            vrow = [pin.tile([128, S], b16, name=f"vrow{kc}") for kc in range(KC)]

            def dma_rows(rows, src, cs):
                for kc in range(KC):
                    nc.sync.dma_start(rows[kc][:, cs], src[kc * 128:(kc + 1) * 128, cs])

            def dma_w4(w_t, src_w):
                # 4 parallel chunks so the weight lands ~4x sooner
                for ch in range(4):
                    cs = slice(ch * 512, (ch + 1) * 512)
                    nc.gpsimd.dma_start(w_t[:, cs], src_w[:, cs])

            q0, q1, half1 = slice(0, 512), slice(512, 1024), slice(1024, 2048)
            # weights on the gpsimd DMA queue, input rows on the sync queue:
            # two descriptor issuers in parallel so first bytes land sooner.
            # chunk 0 needs only the first 512 columns of q/k/v, so those
            # stream first.
            for ch in range(4):
                cs = slice(ch * 512, (ch + 1) * 512)
                nc.gpsimd.dma_start(wq_t[:, cs], wqT[:, cs])
            dma_rows(qrow, qT, q0)
            for ch in range(4):
                cs = slice(ch * 512, (ch + 1) * 512)
                nc.gpsimd.dma_start(wk_t[:, cs], wkT[:, cs])
            nc.gpsimd.dma_start(bq_t[:], bq[:])
            nc.gpsimd.dma_start(bk_t[:], bk[:])
            nc.gpsimd.dma_start(bvb_t[:], bvb[:])
            nc.gpsimd.dma_start(tri_t[:], tri[:])
            nc.gpsimd.dma_start(swp_t[:], swp[:])
            dma_rows(krow, kT, q0)
            dma_w4(wv_t, wvT)
            dma_rows(vrow, vT, q0)
            dma_w4(wo_t, woT)
            dma_rows(qrow, qT, q1)
            dma_rows(krow, kT, q1)
            dma_rows(vrow, vT, q1)
            dma_rows(qrow, qT, half1)
            dma_rows(krow, kT, half1)
            dma_rows(vrow, vT, half1)

            # warmup burst: keeps the PE activity monitor at full clock
            # while the first input quarters stream in
            warm_ps = mmp.tile([128, 1024], f32, tag="sc", name="warm")
            for wi in range(48):
                nc.tensor.matmul(warm_ps[:, 0:128], warm_sb[:], warm_sb[:],
                                 start=(wi == 0), stop=(wi == 47))
            # preload the exp spline tables (~2.7us) during the DMA window
            nc.scalar.activation(warm_sb[:, 0:1], warm_sb[:, 0:1], Exp)

            # --- projections interleaved with attention, per quarter ------
            # (single psum pool assigns slots in emission order, so program
            # order must follow the dataflow for cross-phase overlap)

            def qk_proj_m(name, rows, w_t, dst, bias_t, n, m):
                ps = mmp.tile([128, 1024], f32, tag="sc", name=f"{name}ps{m}{n}")[:, 0:512]
                for kc in range(KC):
                    nc.tensor.matmul(
                        ps,
                        w_t[:, m * D + kc * 128: m * D + (kc + 1) * 128],
                        rows[kc][:, n * 512:(n + 1) * 512],
                        start=(kc == 0), stop=(kc == KC - 1),
                    )
                nc.vector.tensor_scalar_add(
                    dst[:, m * S + n * 512: m * S + (n + 1) * 512],
                    ps, bias_t[:, m:m + 1],
                )

            def qk_proj(name, rows, w_t, dst, bias_t, n):
                for m in range(2):
                    qk_proj_m(name, rows, w_t, dst, bias_t, n, m)

            def v_proj_j(j):
                for j in [j]:
                    psf = mmp.tile([128, 1024], f32, tag="sc", name=f"vps{j}")
                    ps = psf[:, 0:DG]
                    for kc in range(KC):
                        nc.tensor.matmul(
                            ps, vrow[kc][:, j * 128:(j + 1) * 128],
                            wv_t[:, kc * DG:(kc + 1) * DG],
                            start=(kc == 0), stop=(kc == KC - 1),
                        )
                    # copy into the per-head 128-wide slots, adding the v
                    # bias: even heads [vp | ones] (ctx rows 0:64), odd heads
                    # [ones | vp] (ctx rows 64:128) so each head's ctx lands
                    # on the ctxT lanes it must be written to.  One strided
                    # op per parity (heads h and h+2 share offsets mod 256).
                    ps_r = psf.rearrange("p (t y) -> p t y", y=128)
                    vp_r = vp.rearrange("p (t y) -> p t y", y=256)
                    bv_r = bvb_t.rearrange("p (t y) -> p t y", y=128)
                    nc.vector.tensor_add(
                        vp_r[:, 2 * j:2 * j + 2, 0:HD],
                        ps_r[:, 0:2, 0:HD], bv_r[:, 0:2, 0:HD])
                    nc.vector.tensor_add(
                        vp_r[:, 2 * j:2 * j + 2, 128 + HD:256],
                        ps_r[:, 0:2, HD:128], bv_r[:, 0:2, HD:128])

            def v_proj(n):
                for j in range(4 * n, 4 * n + 4):
                    v_proj_j(j)

            def attn_j_sc(c, p, j, nj):
                qoff = p * S + c * 512
                d = j - 4 * c if causal else -1
                coff = 0 if d < 0 else 128 * d
                sc = mmp.tile([128, 1024], f32, tag="sc", name=f"sc{c}{p}{j}")
                for hh, (rlo, rhi) in enumerate(((0, 64), (64, 128))):
                    nc.tensor.matmul(
                        sc[:, hh * 512 + coff: hh * 512 + 512],
                        kpT[rlo:rhi, p * S + j * 128: p * S + (j + 1) * 128],
                        qpT[rlo:rhi, qoff + coff: qoff + 512],
                        start=True, stop=True, tile_position=(rlo, 0),
                    )
                pr = probsp.tile([128, 1024], b16, tag="pr", name=f"pr{c}{p}{j}")
                if coff == 0:
                    nc.scalar.activation(pr[:, 0:1024], sc[:, 0:1024], Exp, scale=1.0 / SCALE)
                else:
                    sc_v = sc.rearrange("p (h n) -> p h n", h=2)[:, :, coff:512]
                    pr_v = pr.rearrange("p (h n) -> p h n", h=2)[:, :, coff:512]
                    nc.scalar.activation(pr_v, sc_v, Exp, scale=1.0 / SCALE)
                if d >= 0:
                    for hh in range(2):
                        band = pr[:, hh * 512 + coff: hh * 512 + coff + 128]
                        nc.vector.tensor_mul(band, band, tri_t[:])
                return pr

            def attn_j_pv(c, p, j, nj, hps, pr):
                d = j - 4 * c if causal else -1
                coff = 0 if d < 0 else 128 * d
                first, last = (j == 0), (j == nj - 1)
                for hh in range(2):
                    h = 2 * p + hh
                    nc.tensor.matmul(
                        hps[:, hh * 512 + coff: hh * 512 + 512],
                        vp[:, j * 512 + h * 128: j * 512 + (h + 1) * 128],
                        pr[:, hh * 512 + coff: hh * 512 + 512],
                        start=first, stop=last, skip_group_check=True,
                    )

            def attn_pair(c, p, nj, hps, bg=None, every=2, lag=3):
                # bg: list of zero-arg emitters (projection groups) woven
                # between attention slots so their psum-slot turns come up
                # mid-stream and their matmuls fill PE slack under the
                # ACT-paced softmax.
                # lag: PV trails the scores by `lag` key tiles so the first
                # PV (which waits for the previous pair's norm to release
                # the shared ctx psum) enters the in-order PE queue late
                # enough not to stall it.
                bg = list(bg or [])
                prs = {}
                for j in range(nj):
                    prs[j] = attn_j_sc(c, p, j, nj)
                    if j >= lag:
                        attn_j_pv(c, p, j - lag, nj, hps, prs.pop(j - lag))
                    if bg and (j + 1) % every == 0:
                        bg.pop(0)()
                # leftover bg first: the trailing PVs may consume v
                # projections still sitting in the bg list
                while bg:
                    bg.pop(0)()
                for j in sorted(prs):
                    attn_j_pv(c, p, j, nj, hps, prs.pop(j))

            def norm_pair(c, p, hps):
                # hps cols 0:512 = even head (ctx rows 0:64, sum rows
                # 64:128); cols 512:1024 = odd head (sum rows 0:64, ctx rows
                # 64:128).  DVE lanes are independent pipelines, so the
                # reciprocal runs on the sum's own lanes and a tiny
                # SBUF->SBUF DMA moves it across lanes to line up with ctx.
                # Evacuate the whole pair psum to SBUF (frees the single cu
                # psum buffer ~1.2us after the last PV), then swap lanes
                # 0:64 <-> 64:128 with a constant permutation matmul so each
                # head's sum lands on its ctx lanes, reciprocal at offset 0,
                # lane-aligned multiplies.  (DVE lanes can't cross; the PE
                # swap costs ~0.4us.)
                st = recp.tile([128, 1024], b16, tag="st", name=f"st{c}{p}")
                nc.vector.tensor_copy(st[:], hps[:])
                sw = mmp.tile([128, 1024], f32, tag="sc", name=f"sw{c}{p}")
                for hh in range(2):
                    nc.tensor.matmul(sw[:, hh * 512:(hh + 1) * 512], swp_t[:],
                                     st[:, hh * 512:(hh + 1) * 512],
                                     start=True, stop=True)
                ra = recp.tile([128, 1024], f32, tag="ra", name=f"ra{c}{p}")
                nc.vector.reciprocal_approx_fast(ra[:], sw[:])
                nc.vector.tensor_mul(
                    ctxT[0:64, p * S + c * 512: p * S + (c + 1) * 512],
                    st[0:64, 0:512], ra[0:64, 0:512])
                nc.vector.tensor_mul(
                    ctxT[64:128, p * S + c * 512: p * S + (c + 1) * 512],
                    st[64:128, 512:1024], ra[64:128, 512:1024])

            def oproj_dc(c, dc):
                for dc in [dc]:
                    ops = mmp.tile([128, 1024], f32, tag="sc", name=f"op{c}{dc}")[:, 0:512]
                    for p2 in range(2):
                        nc.tensor.matmul(
                            ops,
                            wo_t[:, p2 * D + dc * 128: p2 * D + (dc + 1) * 128],
                            ctxT[:, p2 * S + c * 512: p2 * S + (c + 1) * 512],
                            start=(p2 == 0), stop=(p2 == 1),
                        )
                    ot = ostp.tile([128, 512], b16, tag="ot", name=f"ot{c}{dc}")
                    nc.vector.tensor_copy(ot[:], ops)
                    # alternate DMA issue queues so back-to-back output
                    # blocks don't serialize on one sequencer
                    q = nc.sync if dc % 2 == 0 else nc.gpsimd
                    q.dma_start(
                        out[dc * 128:(dc + 1) * 128, c * 512:(c + 1) * 512], ot[:])

            def oproj(c, dcs=range(KC)):
                for dc in dcs:
                    oproj_dc(c, dc)

            qk_proj("q", qrow, wq_t, qpT, bq_t, 0)
            if not causal:
                # no diagonal structure to pipeline against: project all
                # quarters upfront
                for n in range(NQC):
                    if n > 0:
                        qk_proj("q", qrow, wq_t, qpT, bq_t, n)
                    qk_proj("k", krow, wk_t, kpT, bk_t, n)
                    v_proj(n)
            # each pair's norm is deferred into the NEXT pair's weave: its
            # swap matmul would otherwise sit in the in-order PE queue at
            # the pair boundary, stalled on the DVE evacuation copy
            pending_norm = None
            for c in range(NQC):
                nj = 4 * c + 4 if causal else NJ
                hps0 = cup.tile([128, 1024], f32, tag="cu", name=f"cu{c}0")
                # part A (earlier-quarter key tiles; needs only qpT of this
                # quarter) with this quarter's k/v projections woven between
                # slots so they retire before the diagonal part B needs them
                bg_a = []
                if causal:
                    bg_a += [lambda m=m: qk_proj_m("k", krow, wk_t, kpT, bk_t, c, m) for m in range(2)]
                    bg_a += [lambda j=j: v_proj_j(j) for j in range(4 * c, 4 * c + 4)]
                if pending_norm:
                    bg_a.insert(1, pending_norm)
                if c == 0:
                    # no pre-diagonal score tiles to weave against: the k/v
                    # projections must land before the first scores read them
                    while bg_a:
                        bg_a.pop(0)()
                attn_pair(c, 0, nj, hps0, bg=bg_a, every=1)
                # pair 1 hosts: the previous chunk's output projection and
                # the next quarter's q projection
                bg_b = [lambda c=c, h=hps0: norm_pair(c, 0, h)]
                if c > 0:
                    bg_b += [lambda dc=dc: oproj_dc(c - 1, dc) for dc in range(KC)]
                if causal and c + 1 < NQC:
                    bg_b += [lambda m=m: qk_proj_m("q", qrow, wq_t, qpT, bq_t, c + 1, m) for m in range(2)]
                hps1 = cup.tile([128, 1024], f32, tag="cu", name=f"cu{c}1")
                attn_pair(c, 1, nj, hps1, bg=bg_b, every=2)
                pending_norm = (lambda c=c, h=hps1: norm_pair(c, 1, h))
            pending_norm()
            oproj(NQC - 1)

    nc.compile()
    return nc


def _get_nc(causal: bool):
    if causal not in _CACHE:
        _CACHE[causal] = _build(causal)
    return _CACHE[causal]


def _pack_w(w):
    # [D, DG] -> SBUF layout [128, KC*DG]: chunk kc of 128 rows side by side
    return np.ascontiguousarray(w.reshape(KC, 128, DG).transpose(1, 0, 2).reshape(128, KC * DG)).astype(BF16)


def _pack_w_mmajor(w):
    # [D, DG] -> SBUF layout [128, 2*D]: half m of the 256 out dims at
    # cols m*D, within that chunk kc at +kc*128
    return np.ascontiguousarray(
        w.reshape(KC, 128, 2, 128).transpose(1, 2, 0, 3).reshape(128, 2 * D)).astype(BF16)


def make_in_maps(q, k, v, w_q, b_q, w_k, b_k, w_v, b_v, w_o):
    tri_keep = (np.arange(128)[:, None] <= np.arange(128)[None, :]).astype(BF16)
    # lane-swap permutation: out[i] = in[(i + 64) % 128]
    swap_m = np.zeros((128, 128), BF16)
    swap_m[(np.arange(128) + 64) % 128, np.arange(128)] = 1
    qT = [np.ascontiguousarray(q[b].T).astype(BF16) for b in range(B)]
    kTn = [np.ascontiguousarray(k[b].T).astype(BF16) for b in range(B)]
    vTn = [np.ascontiguousarray(v[b].T).astype(BF16) for b in range(B)]
    in_maps = []
    for core in range(NCORES):
        b, g = core // G, core % G
        sl = slice(g * DG, (g + 1) * DG)
        woTg = np.ascontiguousarray(w_o[:, sl].T)  # [DG, D]
        in_maps.append({
            "qT": qT[b], "kT": kTn[b], "vT": vTn[b],
            "wqT": _pack_w_mmajor(np.ascontiguousarray(w_q[sl, :].T)),
            "wkT": _pack_w_mmajor(np.ascontiguousarray(w_k[sl, :].T)),
            "wvT": _pack_w(np.ascontiguousarray(w_v[sl, :].T)),
            "woT": np.ascontiguousarray(
                woTg.reshape(2, 128, D).transpose(1, 0, 2).reshape(128, 2 * D)).astype(BF16),
            "bq": np.ascontiguousarray(b_q[sl].reshape(2, 128).T).astype(np.float32),
            "bk": np.ascontiguousarray(b_k[sl].reshape(2, 128).T).astype(np.float32),
            "bvb": np.ascontiguousarray(np.broadcast_to(b_v[sl], (128, DG))).astype(BF16),
            "tri": tri_keep,
            "swp": swap_m,
        })
    return in_maps


def _reference_numpy(q, k, v, mask, w_q, b_q, w_k, b_k, w_v, b_v, w_o, b_o):
    qp = q @ w_q.T + b_q
    kp = k @ w_k.T + b_k
    vv = v @ w_v.T + b_v
    qp = qp.reshape(B, S, H, HD).transpose(0, 2, 1, 3)
    kp = kp.reshape(B, S, H, HD).transpose(0, 2, 1, 3)
    vv = vv.reshape(B, S, H, HD).transpose(0, 2, 1, 3)
    score = np.einsum("bhqd,bhkd->bhqk", qp, kp) / SCALE
    score = np.where(mask, -1e9, score)
    score -= score.max(axis=-1, keepdims=True)
    e = np.exp(score)
    attn = e / e.sum(axis=-1, keepdims=True)
    ctx = np.einsum("bhqk,bhkd->bhqd", attn, vv)
    ctx = ctx.transpose(0, 2, 1, 3).reshape(B, S, D)
    return (ctx @ w_o.T + b_o).astype(np.float32)


def kernel(q, k, v, mask, w_q, b_q, w_k, b_k, w_v, b_v, w_o, b_o):
    from concourse.bass_utils import run_bass_kernel_spmd

    q, k, v = (np.asarray(x, np.float32) for x in (q, k, v))
    mask = np.asarray(mask)
    causal_ref = np.triu(np.ones((S, S), bool), k=1)
    causal = all(np.array_equal(mask[b, 0], causal_ref) for b in range(B))
    if not causal and mask.any():
        # Unexpected mask pattern: fall back to exact numpy (never hit in
        # practice -- setup_inputs always builds the causal mask).
        return _reference_numpy(q, k, v, mask, w_q, b_q, w_k, b_k, w_v, b_v, w_o, b_o)

    nc = _get_nc(causal)
    in_maps = make_in_maps(q, k, v, w_q, b_q, w_k, b_k, w_v, b_v, w_o)
    res = run_bass_kernel_spmd(nc, in_maps, core_ids=list(range(NCORES)))

    out = np.zeros((B, S, D), np.float32)
    for core in range(NCORES):
        b = core // G
        out[b] += res.results[core]["out"].T.astype(np.float32)
    out += np.asarray(b_o, np.float32)
    return out


# revision 27
# speedup vs baseline: 1.1268x; 1.0386x over previous
"""Multi-head attention (B=2,S=2048,D=1024,H=16) on 8 trn2 NeuronCores.

Sharding: core = b*4 + g  (b = batch 0..1, g = head-group 0..3, 4 heads each).
Each core computes QKV projections for its 256 output dims, causal attention
for its 4 heads (scores kept transposed: [s_k, s_q]), and a K-sliced partial
of the output projection (transposed: [D, S]).  Host sums the 4 partials per
batch and adds b_o.

All matmuls in bf16 (fp32 PSUM accumulate); softmax without max-subtraction
(scores/8 are small, exp cannot overflow).  The PV stationary for head h is
[vp_h | ones] (128 wide): out rows 0:64 accumulate P@V, rows 64:128
accumulate sum(P) broadcast 64-wide -- sumexp costs zero extra PE streams.
Normalization is reciprocal + elementwise multiply on DVE.  v-bias is added
by the DVE copy that moves the v-projection psum into the [vp|ones] layout.
"""
import sys

if "/opt/trn_rl_repo" not in sys.path:
    sys.path.insert(0, "/opt/trn_rl_repo")

import numpy as np
import ml_dtypes

B, S, D, H = 2, 2048, 1024, 16
HD = D // H            # 64
G = 4                  # head groups (one per core within a batch)
HPG = H // G           # 4 heads per group
DG = HPG * HD          # 256 dims per group
SCALE = 8.0
NCORES = 8
NQC = S // 512         # 4 query chunks
NJ = S // 128          # 16 key tiles
KC = D // 128          # 8 contraction chunks
BF16 = ml_dtypes.bfloat16

_CACHE = {}


def _build(causal: bool):
    import concourse.mybir as mybir
    import concourse.tile as tile
    from concourse import bacc

    f32 = mybir.dt.float32
    b16 = mybir.dt.bfloat16
    Exp = mybir.ActivationFunctionType.Exp

    nc = bacc.Bacc(None, target_bir_lowering=False)

    qT = nc.dram_tensor("qT", [D, S], b16, kind="ExternalInput")
    kT = nc.dram_tensor("kT", [D, S], b16, kind="ExternalInput")
    vT = nc.dram_tensor("vT", [D, S], b16, kind="ExternalInput")
    # weights host-prepacked to the exact SBUF tile layout (one DMA each)
    # wq/wk: m-major [128, m*D + kc*128] so the m=0 projection can start
    # after only the first half of the weight lands
    wqT = nc.dram_tensor("wqT", [128, 2 * D], b16, kind="ExternalInput")
    wkT = nc.dram_tensor("wkT", [128, 2 * D], b16, kind="ExternalInput")
    wvT = nc.dram_tensor("wvT", [128, KC * DG], b16, kind="ExternalInput")
    woT = nc.dram_tensor("woT", [128, 2 * D], b16, kind="ExternalInput")
    bq = nc.dram_tensor("bq", [128, 2], f32, kind="ExternalInput")
    bk = nc.dram_tensor("bk", [128, 2], f32, kind="ExternalInput")
    bvb = nc.dram_tensor("bvb", [128, DG], b16, kind="ExternalInput")
    tri = nc.dram_tensor("tri", [128, 128], b16, kind="ExternalInput")
    swp = nc.dram_tensor("swp", [128, 128], b16, kind="ExternalInput")
    out = nc.dram_tensor("out", [D, S], b16, kind="ExternalOutput")

    with tile.TileContext(nc) as tc:
        with (
            tc.tile_pool(name="consts", bufs=1) as consts,
            tc.tile_pool(name="proj", bufs=1) as proj,
            tc.tile_pool(name="pin", bufs=1) as pin,
            tc.tile_pool(name="probs", bufs=8) as probsp,
            tc.tile_pool(name="rec", bufs=2) as recp,
            tc.tile_pool(name="ost", bufs=3) as ostp,
            tc.tile_pool(name="mm", bufs=3, space="PSUM") as mmp,
            tc.tile_pool(name="cu", bufs=1, space="PSUM") as cup,
        ):
            # --- constant tiles -------------------------------------------
            wq_t = consts.tile([128, 2 * D], b16)
            wk_t = consts.tile([128, 2 * D], b16)
            wv_t = consts.tile([128, KC * DG], b16)
            wo_t = consts.tile([128, 2 * D], b16)
            bq_t = consts.tile([128, 2], f32)
            bk_t = consts.tile([128, 2], f32)
            bvb_t = consts.tile([128, DG], b16)
            tri_t = consts.tile([128, 128], b16)
            swp_t = consts.tile([128, 128], b16)
            warm_sb = consts.tile([128, 128], b16)
            nc.vector.memset(warm_sb[:], 0.0)

            # --- persistent projection outputs ----------------------------
            # qpT/kpT: pair p in cols [p*S,(p+1)*S); rows 0:64 head 2p, 64:128 head 2p+1
            qpT = proj.tile([128, 2 * S], b16)
            kpT = proj.tile([128, 2 * S], b16)
            # vp: key tile j at cols [j*512,(j+1)*512); head h at +h*128:
            # 64 cols projected v, 64 cols 1.0 (sumexp block; order flips
            # with head parity).  Its memset is emitted after the DMA
            # section so no queue has it ahead of latency-critical work.
            vp = proj.tile([128, NJ * 512], b16)
            # ctxT: same pair layout as qpT, normalized attention output (c x s)
            ctxT = proj.tile([128, 2 * S], b16)

            # --- input row tiles + DMA schedule ---------------------------
            # quarter-split column DMAs so compute starts as soon as the
            # first 512 columns of each row land.  Each quarter is ONE
            # 3D-AP DMA (row-block b of the [D,S] source lands at cols
            # b*S): per-DMA issue cost (~0.6us) dominated the ramp when
            # each quarter was 8 separate transfers.
            qrow = pin.tile([128, KC * S], b16, name="qrow")
            krow = pin.tile([128, KC * S], b16, name="krow")
            vrow = pin.tile([128, KC * S], b16, name="vrow")

            def dma_rows(rows, src, cs):
                nc.sync.dma_start(
                    rows.rearrange("p (b s) -> p b s", s=S)[:, :, cs],
                    src.rearrange("(b p) s -> p b s", p=128)[:, :, cs],
                )

            q0, q1, half1 = slice(0, 512), slice(512, 1024), slice(1024, 2048)
            # weights on the gpsimd DMA queue, input rows on the sync queue:
            # two descriptor issuers in parallel so first bytes land sooner.
            # chunk 0 needs only the first 512 columns of q/k/v, so those
            # stream first.
            for half in range(2):
                cs = slice(half * D, (half + 1) * D)
                nc.gpsimd.dma_start(wq_t[:, cs], wqT[:, cs])
            dma_rows(qrow, qT, q0)
            for half in range(2):
                cs = slice(half * D, (half + 1) * D)
                nc.gpsimd.dma_start(wk_t[:, cs], wkT[:, cs])
            nc.gpsimd.dma_start(bq_t[:], bq[:])
            nc.gpsimd.dma_start(bk_t[:], bk[:])
            nc.gpsimd.dma_start(bvb_t[:], bvb[:])
            nc.gpsimd.dma_start(tri_t[:], tri[:])
            nc.gpsimd.dma_start(swp_t[:], swp[:])
            dma_rows(krow, kT, q0)
            nc.gpsimd.dma_start(wv_t[:], wvT[:])
            dma_rows(vrow, vT, q0)
            nc.gpsimd.dma_start(wo_t[:], woT[:])
            dma_rows(qrow, qT, q1)
            dma_rows(krow, kT, q1)
            dma_rows(vrow, vT, q1)
            dma_rows(qrow, qT, half1)
            dma_rows(krow, kT, half1)
            dma_rows(vrow, vT, half1)
            nc.vector.memset(vp[:], 1.0)

            # warmup burst: keeps the PE activity monitor at full clock
            # while the first input quarters stream in
            warm_ps = mmp.tile([128, 1024], f32, tag="sc", name="warm")
            for wi in range(48):
                nc.tensor.matmul(warm_ps[:, 0:128], warm_sb[:], warm_sb[:],
                                 start=(wi == 0), stop=(wi == 47))
            # preload the exp spline tables (~2.7us) during the DMA window
            nc.scalar.activation(warm_sb[:, 0:1], warm_sb[:, 0:1], Exp)

            # --- projections interleaved with attention, per quarter ------
            # (single psum pool assigns slots in emission order, so program
            # order must follow the dataflow for cross-phase overlap)

            def qk_proj_m(name, rows, w_t, dst, bias_t, n, m):
                ps = mmp.tile([128, 1024], f32, tag="sc", name=f"{name}ps{m}{n}")[:, 0:512]
                for kc in range(KC):
                    nc.tensor.matmul(
                        ps,
                        w_t[:, m * D + kc * 128: m * D + (kc + 1) * 128],
                        rows[:, kc * S + n * 512: kc * S + (n + 1) * 512],
                        start=(kc == 0), stop=(kc == KC - 1),
                    )
                nc.vector.tensor_scalar_add(
                    dst[:, m * S + n * 512: m * S + (n + 1) * 512],
                    ps, bias_t[:, m:m + 1],
                )

            def qk_proj(name, rows, w_t, dst, bias_t, n):
                for m in range(2):
                    qk_proj_m(name, rows, w_t, dst, bias_t, n, m)

            def v_proj_j(j):
                for j in [j]:
                    psf = mmp.tile([128, 1024], f32, tag="sc", name=f"vps{j}")
                    ps = psf[:, 0:DG]
                    for kc in range(KC):
                        nc.tensor.matmul(
                            ps, vrow[:, kc * S + j * 128: kc * S + (j + 1) * 128],
                            wv_t[:, kc * DG:(kc + 1) * DG],
                            start=(kc == 0), stop=(kc == KC - 1),
                        )
                    # copy into the per-head 128-wide slots, adding the v
                    # bias: even heads [vp | ones] (ctx rows 0:64), odd heads
                    # [ones | vp] (ctx rows 64:128) so each head's ctx lands
                    # on the ctxT lanes it must be written to.  One strided
                    # op per parity (heads h and h+2 share offsets mod 256).
                    ps_r = psf.rearrange("p (t y) -> p t y", y=128)
                    vp_r = vp.rearrange("p (t y) -> p t y", y=256)
                    bv_r = bvb_t.rearrange("p (t y) -> p t y", y=128)
                    nc.vector.tensor_add(
                        vp_r[:, 2 * j:2 * j + 2, 0:HD],
                        ps_r[:, 0:2, 0:HD], bv_r[:, 0:2, 0:HD])
                    nc.vector.tensor_add(
                        vp_r[:, 2 * j:2 * j + 2, 128 + HD:256],
                        ps_r[:, 0:2, HD:128], bv_r[:, 0:2, HD:128])

            def v_proj(n):
                for j in range(4 * n, 4 * n + 4):
                    v_proj_j(j)

            def attn_j_sc(c, p, j, nj):
                qoff = p * S + c * 512
                d = j - 4 * c if causal else -1
                coff = 0 if d < 0 else 128 * d
                sc = mmp.tile([128, 1024], f32, tag="sc", name=f"sc{c}{p}{j}")
                for hh, (rlo, rhi) in enumerate(((0, 64), (64, 128))):
                    nc.tensor.matmul(
                        sc[:, hh * 512 + coff: hh * 512 + 512],
                        kpT[rlo:rhi, p * S + j * 128: p * S + (j + 1) * 128],
                        qpT[rlo:rhi, qoff + coff: qoff + 512],
                        start=True, stop=True, tile_position=(rlo, 0),
                    )
                pr = probsp.tile([128, 1024], b16, tag="pr", name=f"pr{c}{p}{j}")
                if coff == 0:
                    nc.scalar.activation(pr[:, 0:1024], sc[:, 0:1024], Exp, scale=1.0 / SCALE)
                else:
                    sc_v = sc.rearrange("p (h n) -> p h n", h=2)[:, :, coff:512]
                    pr_v = pr.rearrange("p (h n) -> p h n", h=2)[:, :, coff:512]
                    nc.scalar.activation(pr_v, sc_v, Exp, scale=1.0 / SCALE)
                if d >= 0:
                    for hh in range(2):
                        band = pr[:, hh * 512 + coff: hh * 512 + coff + 128]
                        nc.vector.tensor_mul(band, band, tri_t[:])
                return pr

            def attn_j_pv(c, p, j, nj, hps, pr):
                d = j - 4 * c if causal else -1
                coff = 0 if d < 0 else 128 * d
                first, last = (j == 0), (j == nj - 1)
                for hh in range(2):
                    h = 2 * p + hh
                    nc.tensor.matmul(
                        hps[:, hh * 512 + coff: hh * 512 + 512],
                        vp[:, j * 512 + h * 128: j * 512 + (h + 1) * 128],
                        pr[:, hh * 512 + coff: hh * 512 + 512],
                        start=first, stop=last, skip_group_check=True,
                    )

            def attn_pair(c, p, nj, hps, bg=None, every=2, lag=3):
                # bg: list of zero-arg emitters (projection groups) woven
                # between attention slots so their psum-slot turns come up
                # mid-stream and their matmuls fill PE slack under the
                # ACT-paced softmax.
                # lag: PV trails the scores by `lag` key tiles so the first
                # PV (which waits for the previous pair's norm to release
                # the shared ctx psum) enters the in-order PE queue late
                # enough not to stall it.
                bg = list(bg or [])
                prs = {}
                for j in range(nj):
                    prs[j] = attn_j_sc(c, p, j, nj)
                    if j >= lag:
                        attn_j_pv(c, p, j - lag, nj, hps, prs.pop(j - lag))
                    if bg and (j + 1) % every == 0:
                        bg.pop(0)()
                # leftover bg first: the trailing PVs may consume v
                # projections still sitting in the bg list
                while bg:
                    bg.pop(0)()
                for j in sorted(prs):
                    attn_j_pv(c, p, j, nj, hps, prs.pop(j))

            def norm_pair(c, p, hps):
                # hps cols 0:512 = even head (ctx rows 0:64, sum rows
                # 64:128); cols 512:1024 = odd head (sum rows 0:64, ctx rows
                # 64:128).  DVE lanes are independent pipelines, so the
                # reciprocal runs on the sum's own lanes and a tiny
                # SBUF->SBUF DMA moves it across lanes to line up with ctx.
                # Evacuate the whole pair psum to SBUF (frees the single cu
                # psum buffer ~1.2us after the last PV), then swap lanes
                # 0:64 <-> 64:128 with a constant permutation matmul so each
                # head's sum lands on its ctx lanes, reciprocal at offset 0,
                # lane-aligned multiplies.  (DVE lanes can't cross; the PE
                # swap costs ~0.4us.)
                st = recp.tile([128, 1024], b16, tag="st", name=f"st{c}{p}")
                nc.vector.tensor_copy(st[:], hps[:])
                sw = mmp.tile([128, 1024], f32, tag="sc", name=f"sw{c}{p}")
                for hh in range(2):
                    nc.tensor.matmul(sw[:, hh * 512:(hh + 1) * 512], swp_t[:],
                                     st[:, hh * 512:(hh + 1) * 512],
                                     start=True, stop=True)
                ra = recp.tile([128, 1024], f32, tag="ra", name=f"ra{c}{p}")
                nc.vector.reciprocal_approx_fast(ra[:], sw[:])
                nc.vector.tensor_mul(
                    ctxT[0:64, p * S + c * 512: p * S + (c + 1) * 512],
                    st[0:64, 0:512], ra[0:64, 0:512])
                nc.vector.tensor_mul(
                    ctxT[64:128, p * S + c * 512: p * S + (c + 1) * 512],
                    st[64:128, 512:1024], ra[64:128, 512:1024])

            def oproj_dc(c, dc):
                for dc in [dc]:
                    ops = mmp.tile([128, 1024], f32, tag="sc", name=f"op{c}{dc}")[:, 0:512]
                    for p2 in range(2):
                        nc.tensor.matmul(
                            ops,
                            wo_t[:, p2 * D + dc * 128: p2 * D + (dc + 1) * 128],
                            ctxT[:, p2 * S + c * 512: p2 * S + (c + 1) * 512],
                            start=(p2 == 0), stop=(p2 == 1),
                        )
                    ot = ostp.tile([128, 512], b16, tag="ot", name=f"ot{c}{dc}")
                    nc.vector.tensor_copy(ot[:], ops)
                    # alternate DMA issue queues so back-to-back output
                    # blocks don't serialize on one sequencer
                    q = nc.sync if dc % 2 == 0 else nc.gpsimd
                    q.dma_start(
                        out[dc * 128:(dc + 1) * 128, c * 512:(c + 1) * 512], ot[:])

            def oproj(c, dcs=range(KC)):
                for dc in dcs:
                    oproj_dc(c, dc)

            qk_proj("q", qrow, wq_t, qpT, bq_t, 0)
            if not causal:
                # no diagonal structure to pipeline against: project all
                # quarters upfront
                for n in range(NQC):
                    if n > 0:
                        qk_proj("q", qrow, wq_t, qpT, bq_t, n)
                    qk_proj("k", krow, wk_t, kpT, bk_t, n)
                    v_proj(n)
            # each pair's norm is deferred into the NEXT pair's weave: its
            # swap matmul would otherwise sit in the in-order PE queue at
            # the pair boundary, stalled on the DVE evacuation copy
            pending_norm = None
            for c in range(NQC):
                nj = 4 * c + 4 if causal else NJ
                hps0 = cup.tile([128, 1024], f32, tag="cu", name=f"cu{c}0")
                # part A (earlier-quarter key tiles; needs only qpT of this
                # quarter) with this quarter's k/v projections woven between
                # slots so they retire before the diagonal part B needs them
                bg_a = []
                if causal:
                    bg_a += [lambda m=m: qk_proj_m("k", krow, wk_t, kpT, bk_t, c, m) for m in range(2)]
                    bg_a += [lambda j=j: v_proj_j(j) for j in range(4 * c, 4 * c + 4)]
                if pending_norm:
                    bg_a.insert(1, pending_norm)
                if c == 0:
                    # no pre-diagonal score tiles to weave against: the k/v
                    # projections must land before the first scores read them
                    while bg_a:
                        bg_a.pop(0)()
                attn_pair(c, 0, nj, hps0, bg=bg_a, every=1)
                # pair 1 hosts: the previous chunk's output projection and
                # the next quarter's q projection
                bg_b = [lambda c=c, h=hps0: norm_pair(c, 0, h)]
                if c > 0:
                    bg_b += [lambda dc=dc: oproj_dc(c - 1, dc) for dc in range(KC)]
                if causal and c + 1 < NQC:
                    bg_b += [lambda m=m: qk_proj_m("q", qrow, wq_t, qpT, bq_t, c + 1, m) for m in range(2)]
                hps1 = cup.tile([128, 1024], f32, tag="cu", name=f"cu{c}1")
                attn_pair(c, 1, nj, hps1, bg=bg_b, every=2)
                pending_norm = (lambda c=c, h=hps1: norm_pair(c, 1, h))
            pending_norm()
            oproj(NQC - 1)

    nc.compile()
    return nc


def _get_nc(causal: bool):
    if causal not in _CACHE:
        _CACHE[causal] = _build(causal)
    return _CACHE[causal]


def _pack_w(w):
    # [D, DG] -> SBUF layout [128, KC*DG]: chunk kc of 128 rows side by side
    return np.ascontiguousarray(w.reshape(KC, 128, DG).transpose(1, 0, 2).reshape(128, KC * DG)).astype(BF16)


def _pack_w_mmajor(w):
    # [D, DG] -> SBUF layout [128, 2*D]: half m of the 256 out dims at
    # cols m*D, within that chunk kc at +kc*128
    return np.ascontiguousarray(
        w.reshape(KC, 128, 2, 128).transpose(1, 2, 0, 3).reshape(128, 2 * D)).astype(BF16)


def make_in_maps(q, k, v, w_q, b_q, w_k, b_k, w_v, b_v, w_o):
    tri_keep = (np.arange(128)[:, None] <= np.arange(128)[None, :]).astype(BF16)
    # lane-swap permutation: out[i] = in[(i + 64) % 128]
    swap_m = np.zeros((128, 128), BF16)
    swap_m[(np.arange(128) + 64) % 128, np.arange(128)] = 1
    qT = [np.ascontiguousarray(q[b].T).astype(BF16) for b in range(B)]
    kTn = [np.ascontiguousarray(k[b].T).astype(BF16) for b in range(B)]
    vTn = [np.ascontiguousarray(v[b].T).astype(BF16) for b in range(B)]
    in_maps = []
    for core in range(NCORES):
        b, g = core // G, core % G
        sl = slice(g * DG, (g + 1) * DG)
        woTg = np.ascontiguousarray(w_o[:, sl].T)  # [DG, D]
        in_maps.append({
            "qT": qT[b], "kT": kTn[b], "vT": vTn[b],
            "wqT": _pack_w_mmajor(np.ascontiguousarray(w_q[sl, :].T)),
            "wkT": _pack_w_mmajor(np.ascontiguousarray(w_k[sl, :].T)),
            "wvT": _pack_w(np.ascontiguousarray(w_v[sl, :].T)),
            "woT": np.ascontiguousarray(
                woTg.reshape(2, 128, D).transpose(1, 0, 2).reshape(128, 2 * D)).astype(BF16),
            "bq": np.ascontiguousarray(b_q[sl].reshape(2, 128).T).astype(np.float32),
            "bk": np.ascontiguousarray(b_k[sl].reshape(2, 128).T).astype(np.float32),
            "bvb": np.ascontiguousarray(np.broadcast_to(b_v[sl], (128, DG))).astype(BF16),
            "tri": tri_keep,
            "swp": swap_m,
        })
    return in_maps


def _reference_numpy(q, k, v, mask, w_q, b_q, w_k, b_k, w_v, b_v, w_o, b_o):
    qp = q @ w_q.T + b_q
    kp = k @ w_k.T + b_k
    vv = v @ w_v.T + b_v
    qp = qp.reshape(B, S, H, HD).transpose(0, 2, 1, 3)
    kp = kp.reshape(B, S, H, HD).transpose(0, 2, 1, 3)
    vv = vv.reshape(B, S, H, HD).transpose(0, 2, 1, 3)
    score = np.einsum("bhqd,bhkd->bhqk", qp, kp) / SCALE
    score = np.where(mask, -1e9, score)
    score -= score.max(axis=-1, keepdims=True)
    e = np.exp(score)
    attn = e / e.sum(axis=-1, keepdims=True)
    ctx = np.einsum("bhqk,bhkd->bhqd", attn, vv)
    ctx = ctx.transpose(0, 2, 1, 3).reshape(B, S, D)
    return (ctx @ w_o.T + b_o).astype(np.float32)


def kernel(q, k, v, mask, w_q, b_q, w_k, b_k, w_v, b_v, w_o, b_o):
    from concourse.bass_utils import run_bass_kernel_spmd

    q, k, v = (np.asarray(x, np.float32) for x in (q, k, v))
    mask = np.asarray(mask)
    causal_ref = np.triu(np.ones((S, S), bool), k=1)
    causal = all(np.array_equal(mask[b, 0], causal_ref) for b in range(B))
    if not causal and mask.any():
        # Unexpected mask pattern: fall back to exact numpy (never hit in
        # practice -- setup_inputs always builds the causal mask).
        return _reference_numpy(q, k, v, mask, w_q, b_q, w_k, b_k, w_v, b_v, w_o, b_o)

    nc = _get_nc(causal)
    in_maps = make_in_maps(q, k, v, w_q, b_q, w_k, b_k, w_v, b_v, w_o)
    res = run_bass_kernel_spmd(nc, in_maps, core_ids=list(range(NCORES)))

    out = np.zeros((B, S, D), np.float32)
    for core in range(NCORES):
        b = core // G
        out[b] += res.results[core]["out"].T.astype(np.float32)
    out += np.asarray(b_o, np.float32)
    return out


# revision 29
# speedup vs baseline: 1.1469x; 1.0178x over previous
"""Multi-head attention (B=2,S=2048,D=1024,H=16) on 8 trn2 NeuronCores.

Sharding: core = b*4 + g  (b = batch 0..1, g = head-group 0..3, 4 heads each).
Each core computes QKV projections for its 256 output dims, causal attention
for its 4 heads (scores kept transposed: [s_k, s_q]), and a K-sliced partial
of the output projection (transposed: [D, S]).  Host sums the 4 partials per
batch and adds b_o.

All matmuls in bf16 (fp32 PSUM accumulate); softmax without max-subtraction
(scores/8 are small, exp cannot overflow).  The PV stationary for head h is
[vp_h | ones] (128 wide): out rows 0:64 accumulate P@V, rows 64:128
accumulate sum(P) broadcast 64-wide -- sumexp costs zero extra PE streams.
Normalization is reciprocal + elementwise multiply on DVE.  v-bias is added
by the DVE copy that moves the v-projection psum into the [vp|ones] layout.
"""
import sys

if "/opt/trn_rl_repo" not in sys.path:
    sys.path.insert(0, "/opt/trn_rl_repo")

import numpy as np
import ml_dtypes

B, S, D, H = 2, 2048, 1024, 16
HD = D // H            # 64
G = 4                  # head groups (one per core within a batch)
HPG = H // G           # 4 heads per group
DG = HPG * HD          # 256 dims per group
SCALE = 8.0
NCORES = 8
NQC = S // 512         # 4 query chunks
NJ = S // 128          # 16 key tiles
KC = D // 128          # 8 contraction chunks
BF16 = ml_dtypes.bfloat16

_CACHE = {}


def _build(causal: bool):
    import concourse.mybir as mybir
    import concourse.tile as tile
    from concourse import bacc

    f32 = mybir.dt.float32
    b16 = mybir.dt.bfloat16
    Exp = mybir.ActivationFunctionType.Exp

    nc = bacc.Bacc(None, target_bir_lowering=False)

    qT = nc.dram_tensor("qT", [D, S], b16, kind="ExternalInput")
    kT = nc.dram_tensor("kT", [D, S], b16, kind="ExternalInput")
    vT = nc.dram_tensor("vT", [D, S], b16, kind="ExternalInput")
    # weights host-prepacked to the exact SBUF tile layout (one DMA each)
    # wq/wk: m-major [128, m*D + kc*128] so the m=0 projection can start
    # after only the first half of the weight lands
    wqT = nc.dram_tensor("wqT", [128, 2 * D], b16, kind="ExternalInput")
    wkT = nc.dram_tensor("wkT", [128, 2 * D], b16, kind="ExternalInput")
    wvT = nc.dram_tensor("wvT", [128, KC * DG], b16, kind="ExternalInput")
    woT = nc.dram_tensor("woT", [128, 2 * D], b16, kind="ExternalInput")
    bq = nc.dram_tensor("bq", [128, 2], f32, kind="ExternalInput")
    bk = nc.dram_tensor("bk", [128, 2], f32, kind="ExternalInput")
    bvb = nc.dram_tensor("bvb", [128, DG], b16, kind="ExternalInput")
    tri = nc.dram_tensor("tri", [128, 128], b16, kind="ExternalInput")
    swp = nc.dram_tensor("swp", [128, 128], b16, kind="ExternalInput")
    out = nc.dram_tensor("out", [D, S], b16, kind="ExternalOutput")

    with tile.TileContext(nc) as tc:
        with (
            tc.tile_pool(name="consts", bufs=1) as consts,
            tc.tile_pool(name="proj", bufs=1) as proj,
            tc.tile_pool(name="pin", bufs=1) as pin,
            tc.tile_pool(name="probs", bufs=8) as probsp,
            tc.tile_pool(name="rec", bufs=2) as recp,
            tc.tile_pool(name="ost", bufs=3) as ostp,
            tc.tile_pool(name="mm", bufs=3, space="PSUM") as mmp,
            tc.tile_pool(name="cu", bufs=1, space="PSUM") as cup,
        ):
            # --- constant tiles -------------------------------------------
            wq_t = consts.tile([128, 2 * D], b16)
            wk_t = consts.tile([128, 2 * D], b16)
            wv_t = consts.tile([128, KC * DG], b16)
            wo_t = consts.tile([128, 2 * D], b16)
            bq_t = consts.tile([128, 2], f32)
            bk_t = consts.tile([128, 2], f32)
            bvb_t = consts.tile([128, DG], b16)
            tri_t = consts.tile([128, 128], b16)
            swp_t = consts.tile([128, 128], b16)
            warm_sb = consts.tile([128, 128], b16)
            nc.vector.memset(warm_sb[:], 0.0)

            # --- persistent projection outputs ----------------------------
            # qpT/kpT: pair p in cols [p*S,(p+1)*S); rows 0:64 head 2p, 64:128 head 2p+1
            qpT = proj.tile([128, 2 * S], b16)
            kpT = proj.tile([128, 2 * S], b16)
            # vp: key tile j at cols [j*512,(j+1)*512); head h at +h*128:
            # 64 cols projected v, 64 cols 1.0 (sumexp block; order flips
            # with head parity).  Its memset is emitted after the DMA
            # section so no queue has it ahead of latency-critical work.
            vp = proj.tile([128, NJ * 512], b16)
            # ctxT: same pair layout as qpT, normalized attention output (c x s)
            ctxT = proj.tile([128, 2 * S], b16)

            # --- input row tiles + DMA schedule ---------------------------
            # quarter-split column DMAs so compute starts as soon as the
            # first 512 columns of each row land.  Each quarter is ONE
            # 3D-AP DMA (row-block b of the [D,S] source lands at cols
            # b*S): per-DMA issue cost (~0.6us) dominated the ramp when
            # each quarter was 8 separate transfers.
            qrow = pin.tile([128, KC * S], b16, name="qrow")
            krow = pin.tile([128, KC * S], b16, name="krow")
            vrow = pin.tile([128, KC * S], b16, name="vrow")

            def dma_rows(rows, src, cs):
                nc.sync.dma_start(
                    rows.rearrange("p (b s) -> p b s", s=S)[:, :, cs],
                    src.rearrange("(b p) s -> p b s", p=128)[:, :, cs],
                )

            q0, q1, half1 = slice(0, 512), slice(512, 1024), slice(1024, 2048)
            # weights on the gpsimd DMA queue, input rows on the sync queue:
            # two descriptor issuers in parallel so first bytes land sooner.
            # chunk 0 needs only the first 512 columns of q/k/v, so those
            # stream first.
            for half in range(2):
                cs = slice(half * D, (half + 1) * D)
                nc.gpsimd.dma_start(wq_t[:, cs], wqT[:, cs])
            dma_rows(qrow, qT, q0)
            for half in range(2):
                cs = slice(half * D, (half + 1) * D)
                nc.gpsimd.dma_start(wk_t[:, cs], wkT[:, cs])
            nc.gpsimd.dma_start(bq_t[:], bq[:])
            nc.gpsimd.dma_start(bk_t[:], bk[:])
            nc.gpsimd.dma_start(bvb_t[:], bvb[:])
            nc.gpsimd.dma_start(tri_t[:], tri[:])
            nc.gpsimd.dma_start(swp_t[:], swp[:])
            dma_rows(krow, kT, q0)
            dma_rows(vrow, vT, q0)
            nc.sync.dma_start(wv_t[:], wvT[:])
            nc.gpsimd.dma_start(wo_t[:], woT[:])
            dma_rows(qrow, qT, q1)
            dma_rows(krow, kT, q1)
            dma_rows(vrow, vT, q1)
            dma_rows(qrow, qT, half1)
            dma_rows(krow, kT, half1)
            dma_rows(vrow, vT, half1)
            nc.vector.memset(vp[:], 1.0)

            # warmup burst: keeps the PE activity monitor at full clock
            # while the first input quarters stream in
            warm_ps = mmp.tile([128, 1024], f32, tag="sc", name="warm")
            for wi in range(36):
                nc.tensor.matmul(warm_ps[:, 0:128], warm_sb[:], warm_sb[:],
                                 start=(wi == 0), stop=(wi == 35))
            # preload the exp spline tables (~2.7us) during the DMA window
            nc.scalar.activation(warm_sb[:, 0:1], warm_sb[:, 0:1], Exp)

            # --- projections interleaved with attention, per quarter ------
            # (single psum pool assigns slots in emission order, so program
            # order must follow the dataflow for cross-phase overlap)

            def qk_proj_m(name, rows, w_t, dst, bias_t, n, m):
                ps = mmp.tile([128, 1024], f32, tag="sc", name=f"{name}ps{m}{n}")[:, 0:512]
                for kc in range(KC):
                    nc.tensor.matmul(
                        ps,
                        w_t[:, m * D + kc * 128: m * D + (kc + 1) * 128],
                        rows[:, kc * S + n * 512: kc * S + (n + 1) * 512],
                        start=(kc == 0), stop=(kc == KC - 1),
                    )
                nc.vector.tensor_scalar_add(
                    dst[:, m * S + n * 512: m * S + (n + 1) * 512],
                    ps, bias_t[:, m:m + 1],
                )

            def qk_proj(name, rows, w_t, dst, bias_t, n):
                for m in range(2):
                    qk_proj_m(name, rows, w_t, dst, bias_t, n, m)

            def v_proj_j(j):
                for j in [j]:
                    psf = mmp.tile([128, 1024], f32, tag="sc", name=f"vps{j}")
                    ps = psf[:, 0:DG]
                    for kc in range(KC):
                        nc.tensor.matmul(
                            ps, vrow[:, kc * S + j * 128: kc * S + (j + 1) * 128],
                            wv_t[:, kc * DG:(kc + 1) * DG],
                            start=(kc == 0), stop=(kc == KC - 1),
                        )
                    # copy into the per-head 128-wide slots, adding the v
                    # bias: even heads [vp | ones] (ctx rows 0:64), odd heads
                    # [ones | vp] (ctx rows 64:128) so each head's ctx lands
                    # on the ctxT lanes it must be written to.  One strided
                    # op per parity (heads h and h+2 share offsets mod 256).
                    ps_r = psf.rearrange("p (t y) -> p t y", y=128)
                    vp_r = vp.rearrange("p (t y) -> p t y", y=256)
                    bv_r = bvb_t.rearrange("p (t y) -> p t y", y=128)
                    nc.vector.tensor_add(
                        vp_r[:, 2 * j:2 * j + 2, 0:HD],
                        ps_r[:, 0:2, 0:HD], bv_r[:, 0:2, 0:HD])
                    nc.vector.tensor_add(
                        vp_r[:, 2 * j:2 * j + 2, 128 + HD:256],
                        ps_r[:, 0:2, HD:128], bv_r[:, 0:2, HD:128])

            def v_proj(n):
                for j in range(4 * n, 4 * n + 4):
                    v_proj_j(j)

            def attn_j_sc(c, p, j, nj):
                qoff = p * S + c * 512
                d = j - 4 * c if causal else -1
                coff = 0 if d < 0 else 128 * d
                sc = mmp.tile([128, 1024], f32, tag="sc", name=f"sc{c}{p}{j}")
                for hh, (rlo, rhi) in enumerate(((0, 64), (64, 128))):
                    nc.tensor.matmul(
                        sc[:, hh * 512 + coff: hh * 512 + 512],
                        kpT[rlo:rhi, p * S + j * 128: p * S + (j + 1) * 128],
                        qpT[rlo:rhi, qoff + coff: qoff + 512],
                        start=True, stop=True, tile_position=(rlo, 0),
                    )
                pr = probsp.tile([128, 1024], b16, tag="pr", name=f"pr{c}{p}{j}")
                if coff == 0:
                    nc.scalar.activation(pr[:, 0:1024], sc[:, 0:1024], Exp, scale=1.0 / SCALE)
                else:
                    sc_v = sc.rearrange("p (h n) -> p h n", h=2)[:, :, coff:512]
                    pr_v = pr.rearrange("p (h n) -> p h n", h=2)[:, :, coff:512]
                    nc.scalar.activation(pr_v, sc_v, Exp, scale=1.0 / SCALE)
                if d >= 0:
                    for hh in range(2):
                        band = pr[:, hh * 512 + coff: hh * 512 + coff + 128]
                        nc.vector.tensor_mul(band, band, tri_t[:])
                return pr

            def attn_j_pv(c, p, j, nj, hps, pr):
                d = j - 4 * c if causal else -1
                coff = 0 if d < 0 else 128 * d
                first, last = (j == 0), (j == nj - 1)
                for hh in range(2):
                    h = 2 * p + hh
                    nc.tensor.matmul(
                        hps[:, hh * 512 + coff: hh * 512 + 512],
                        vp[:, j * 512 + h * 128: j * 512 + (h + 1) * 128],
                        pr[:, hh * 512 + coff: hh * 512 + 512],
                        start=first, stop=last, skip_group_check=True,
                    )

            def attn_pair(c, p, nj, hps, bg=None, every=2, lag=3):
                # bg: list of zero-arg emitters (projection groups) woven
                # between attention slots so their psum-slot turns come up
                # mid-stream and their matmuls fill PE slack under the
                # ACT-paced softmax.
                # lag: PV trails the scores by `lag` key tiles so the first
                # PV (which waits for the previous pair's norm to release
                # the shared ctx psum) enters the in-order PE queue late
                # enough not to stall it.
                bg = list(bg or [])
                prs = {}
                for j in range(nj):
                    prs[j] = attn_j_sc(c, p, j, nj)
                    if j >= lag:
                        attn_j_pv(c, p, j - lag, nj, hps, prs.pop(j - lag))
                    if bg and (j + 1) % every == 0:
                        bg.pop(0)()
                # leftover bg first: the trailing PVs may consume v
                # projections still sitting in the bg list
                while bg:
                    bg.pop(0)()
                for j in sorted(prs):
                    attn_j_pv(c, p, j, nj, hps, prs.pop(j))

            # Normalization, two phases.  evac_pair: copy the pair psum
            # (even head: ctx rows 0:64, sum rows 64:128 in cols 0:512;
            # odd head mirrored in cols 512:1024) to SBUF right at pair
            # end -- frees the single cu psum buffer ~1.2us after the last
            # PV without touching the PE queue.  norm_rest: swap lanes
            # 0:64 <-> 64:128 with a constant permutation matmul so each
            # head's sum lands on its ctx lanes (DVE lanes cannot cross),
            # reciprocal at offset 0, lane-aligned multiplies -- woven a
            # few key tiles into the NEXT pair so the swap matmul never
            # stalls the in-order PE queue waiting for the copy.
            def evac_pair(c, p, hps):
                st = recp.tile([128, 1024], b16, tag="st", name=f"st{c}{p}")
                nc.vector.tensor_copy(st[:], hps[:])
                return st

            def norm_rest(c, p, st):
                sw = mmp.tile([128, 1024], f32, tag="sc", name=f"sw{c}{p}")
                for hh in range(2):
                    nc.tensor.matmul(sw[:, hh * 512:(hh + 1) * 512], swp_t[:],
                                     st[:, hh * 512:(hh + 1) * 512],
                                     start=True, stop=True)
                ra = recp.tile([128, 1024], f32, tag="ra", name=f"ra{c}{p}")
                nc.vector.reciprocal_approx_fast(ra[:], sw[:])
                nc.vector.tensor_mul(
                    ctxT[0:64, p * S + c * 512: p * S + (c + 1) * 512],
                    st[0:64, 0:512], ra[0:64, 0:512])
                nc.vector.tensor_mul(
                    ctxT[64:128, p * S + c * 512: p * S + (c + 1) * 512],
                    st[64:128, 512:1024], ra[64:128, 512:1024])

            def oproj_dc(c, dc):
                for dc in [dc]:
                    ops = mmp.tile([128, 1024], f32, tag="sc", name=f"op{c}{dc}")[:, 0:512]
                    for p2 in range(2):
                        nc.tensor.matmul(
                            ops,
                            wo_t[:, p2 * D + dc * 128: p2 * D + (dc + 1) * 128],
                            ctxT[:, p2 * S + c * 512: p2 * S + (c + 1) * 512],
                            start=(p2 == 0), stop=(p2 == 1),
                        )
                    ot = ostp.tile([128, 512], b16, tag="ot", name=f"ot{c}{dc}")
                    nc.vector.tensor_copy(ot[:], ops)
                    # alternate DMA issue queues so back-to-back output
                    # blocks don't serialize on one sequencer
                    q = nc.sync if dc % 2 == 0 else nc.gpsimd
                    q.dma_start(
                        out[dc * 128:(dc + 1) * 128, c * 512:(c + 1) * 512], ot[:])

            def oproj(c, dcs=range(KC)):
                for dc in dcs:
                    oproj_dc(c, dc)

            qk_proj("q", qrow, wq_t, qpT, bq_t, 0)
            if not causal:
                # no diagonal structure to pipeline against: project all
                # quarters upfront
                for n in range(NQC):
                    if n > 0:
                        qk_proj("q", qrow, wq_t, qpT, bq_t, n)
                    qk_proj("k", krow, wk_t, kpT, bk_t, n)
                    v_proj(n)
            # each pair ends with its psum evacuation (DVE, emitted
            # immediately); the rest of its norm is deferred into the NEXT
            # pair's weave so the swap matmul enters the in-order PE queue
            # only after the evacuation has long completed
            pending_norm = None
            for c in range(NQC):
                nj = 4 * c + 4 if causal else NJ
                hps0 = cup.tile([128, 1024], f32, tag="cu", name=f"cu{c}0")
                # part A (earlier-quarter key tiles; needs only qpT of this
                # quarter) with this quarter's k/v projections woven between
                # slots so they retire before the diagonal part B needs them
                bg_a = []
                if causal:
                    bg_a += [lambda m=m: qk_proj_m("k", krow, wk_t, kpT, bk_t, c, m) for m in range(2)]
                    bg_a += [lambda j=j: v_proj_j(j) for j in range(4 * c, 4 * c + 4)]
                if pending_norm:
                    bg_a.insert(2, pending_norm)
                if c == 0:
                    # no pre-diagonal score tiles to weave against: the k/v
                    # projections must land before the first scores read them
                    while bg_a:
                        bg_a.pop(0)()
                attn_pair(c, 0, nj, hps0, bg=bg_a, every=1)
                st0 = evac_pair(c, 0, hps0)
                # pair 1 hosts: the previous chunk's output projection and
                # the next quarter's q projection
                bg_b = [lambda dc=0: oproj_dc(c - 1, 0)] if c > 0 else []
                bg_b += [lambda c=c, s=st0: norm_rest(c, 0, s)]
                if c > 0:
                    bg_b += [lambda dc=dc: oproj_dc(c - 1, dc) for dc in range(1, KC)]
                if causal and c + 1 < NQC:
                    bg_b += [lambda m=m: qk_proj_m("q", qrow, wq_t, qpT, bq_t, c + 1, m) for m in range(2)]
                hps1 = cup.tile([128, 1024], f32, tag="cu", name=f"cu{c}1")
                attn_pair(c, 1, nj, hps1, bg=bg_b, every=2)
                st1 = evac_pair(c, 1, hps1)
                pending_norm = (lambda c=c, s=st1: norm_rest(c, 1, s))
            pending_norm()
            oproj(NQC - 1)

    nc.compile()
    return nc


def _get_nc(causal: bool):
    if causal not in _CACHE:
        _CACHE[causal] = _build(causal)
    return _CACHE[causal]


def _pack_w(w):
    # [D, DG] -> SBUF layout [128, KC*DG]: chunk kc of 128 rows side by side
    return np.ascontiguousarray(w.reshape(KC, 128, DG).transpose(1, 0, 2).reshape(128, KC * DG)).astype(BF16)


def _pack_w_mmajor(w):
    # [D, DG] -> SBUF layout [128, 2*D]: half m of the 256 out dims at
    # cols m*D, within that chunk kc at +kc*128
    return np.ascontiguousarray(
        w.reshape(KC, 128, 2, 128).transpose(1, 2, 0, 3).reshape(128, 2 * D)).astype(BF16)


def make_in_maps(q, k, v, w_q, b_q, w_k, b_k, w_v, b_v, w_o):
    tri_keep = (np.arange(128)[:, None] <= np.arange(128)[None, :]).astype(BF16)
    # lane-swap permutation: out[i] = in[(i + 64) % 128]
    swap_m = np.zeros((128, 128), BF16)
    swap_m[(np.arange(128) + 64) % 128, np.arange(128)] = 1
    qT = [np.ascontiguousarray(q[b].T).astype(BF16) for b in range(B)]
    kTn = [np.ascontiguousarray(k[b].T).astype(BF16) for b in range(B)]
    vTn = [np.ascontiguousarray(v[b].T).astype(BF16) for b in range(B)]
    in_maps = []
    for core in range(NCORES):
        b, g = core // G, core % G
        sl = slice(g * DG, (g + 1) * DG)
        woTg = np.ascontiguousarray(w_o[:, sl].T)  # [DG, D]
        in_maps.append({
            "qT": qT[b], "kT": kTn[b], "vT": vTn[b],
            "wqT": _pack_w_mmajor(np.ascontiguousarray(w_q[sl, :].T)),
            "wkT": _pack_w_mmajor(np.ascontiguousarray(w_k[sl, :].T)),
            "wvT": _pack_w(np.ascontiguousarray(w_v[sl, :].T)),
            "woT": np.ascontiguousarray(
                woTg.reshape(2, 128, D).transpose(1, 0, 2).reshape(128, 2 * D)).astype(BF16),
            "bq": np.ascontiguousarray(b_q[sl].reshape(2, 128).T).astype(np.float32),
            "bk": np.ascontiguousarray(b_k[sl].reshape(2, 128).T).astype(np.float32),
            "bvb": np.ascontiguousarray(np.broadcast_to(b_v[sl], (128, DG))).astype(BF16),
            "tri": tri_keep,
            "swp": swap_m,
        })
    return in_maps


def _reference_numpy(q, k, v, mask, w_q, b_q, w_k, b_k, w_v, b_v, w_o, b_o):
    qp = q @ w_q.T + b_q
    kp = k @ w_k.T + b_k
    vv = v @ w_v.T + b_v
    qp = qp.reshape(B, S, H, HD).transpose(0, 2, 1, 3)
    kp = kp.reshape(B, S, H, HD).transpose(0, 2, 1, 3)
    vv = vv.reshape(B, S, H, HD).transpose(0, 2, 1, 3)
    score = np.einsum("bhqd,bhkd->bhqk", qp, kp) / SCALE
    score = np.where(mask, -1e9, score)
    score -= score.max(axis=-1, keepdims=True)
    e = np.exp(score)
    attn = e / e.sum(axis=-1, keepdims=True)
    ctx = np.einsum("bhqk,bhkd->bhqd", attn, vv)
    ctx = ctx.transpose(0, 2, 1, 3).reshape(B, S, D)
    return (ctx @ w_o.T + b_o).astype(np.float32)


def kernel(q, k, v, mask, w_q, b_q, w_k, b_k, w_v, b_v, w_o, b_o):
    from concourse.bass_utils import run_bass_kernel_spmd

    q, k, v = (np.asarray(x, np.float32) for x in (q, k, v))
    mask = np.asarray(mask)
    causal_ref = np.triu(np.ones((S, S), bool), k=1)
    causal = all(np.array_equal(mask[b, 0], causal_ref) for b in range(B))
    if not causal and mask.any():
        # Unexpected mask pattern: fall back to exact numpy (never hit in
        # practice -- setup_inputs always builds the causal mask).
        return _reference_numpy(q, k, v, mask, w_q, b_q, w_k, b_k, w_v, b_v, w_o, b_o)

    nc = _get_nc(causal)
    in_maps = make_in_maps(q, k, v, w_q, b_q, w_k, b_k, w_v, b_v, w_o)
    res = run_bass_kernel_spmd(nc, in_maps, core_ids=list(range(NCORES)))

    out = np.zeros((B, S, D), np.float32)
    for core in range(NCORES):
        b = core // G
        out[b] += res.results[core]["out"].T.astype(np.float32)
    out += np.asarray(b_o, np.float32)
    return out
